# revision 77
# baseline (speedup 1.0000x reference)
"""Trainium2 Bass kernel for the CAAM sparse-attention module.

Data-parallel over batch B=8 across 8 NeuronCores (one image per core).
All parameters replicated. Matmul fabric runs in bf16 (fp32 PSUM
accumulation); softmax normalizers, biases and the residual path stay
fp32.

Layouts: x streamed in row-major quarter-bin-rows [512c, 8 rows x 128
cols] (contiguous 512B DMA runs), cast to bf16 in-flight by the gpsimd
DMA. The per-bin pixel contraction (local = pixconf @ x_p) uses full
image-row transposes ([128 px, c]) with a zero-padded block-diagonal
E_T stationary [128 px, (4 bins x 32)] so a single matmul accumulates
all 4 bins of a bin-row into one stacked [128, 512] PSUM (bin j on
partitions 32j..32j+18). The GCN mix emits the same stacked layout.
q is written bin-major so pass-2 attention matmuls see contiguous APs.
"""

import os

os.environ.setdefault("JAX_COMPILATION_CACHE_DIR", "/tmp/jax_comp_cache")
os.environ.setdefault("MYCRO_LOCAL_CACHE", "1")

import numpy as np
import ml_dtypes

import concourse.bass as bass
import concourse.mybir as mybir
import concourse.tile as tile
from contextlib import ExitStack

dt = mybir.dt
F32 = dt.float32
BF16 = dt.bfloat16
FP8 = dt.float8e4
AX = mybir.AxisListType
AF = mybir.ActivationFunctionType
ALU = mybir.AluOpType

C, H, W, K, CI = 512, 128, 128, 19, 256
NBINS = 16          # 4x4 bins
PBIN = 1024         # 32*32 pixels per bin
NCORES = 8


def build_nc():
    nc = bass.Bass("TRN2", target_bir_lowering=False, debug=False)

    x_d = nc.declare_dram_parameter("x", [C, H, W], F32, isOutput=False)
    camw_d = nc.declare_dram_parameter("cam_wT", [128, 4 * K], BF16, isOutput=False)
    camb_d = nc.declare_dram_parameter("cam_b", [K, 1], F32, isOutput=False)
    qw_d = nc.declare_dram_parameter("q_wT", [128, 1024], BF16, isOutput=False)
    kw_d = nc.declare_dram_parameter("k_wT", [128, 1024], BF16, isOutput=False)
    vw_d = nc.declare_dram_parameter("v_wT", [128, 1024], BF16, isOutput=False)
    linw_d = nc.declare_dram_parameter("lin_wT", [128, 2048], BF16, isOutput=False)
    outw_d = nc.declare_dram_parameter("out_w8", [128, 1024], FP8, isOutput=False)
    deq_d = nc.declare_dram_parameter("out_deq", [128, 1], F32, isOutput=False)
    w1s_d = nc.declare_dram_parameter("w1s", [128, 3 * 512], BF16, isOutput=False)
    fuses_d = nc.declare_dram_parameter("fuse_s", [128, 3 * K], BF16, isOutput=False)
    i128_d = nc.declare_dram_parameter("i128", [128, 128], BF16, isOutput=False)
    si19_d = nc.declare_dram_parameter("si19", [128, K], BF16, isOutput=False)
    si76_d = nc.declare_dram_parameter("si76", [128, 76], BF16, isOutput=False)
    outpa_raw_d = nc.declare_dram_parameter("out_pa", [128, 4], F32, isOutput=False)
    ones19_d = nc.declare_dram_parameter("ones19", [K, 1], BF16, isOutput=False)
    ones1_d = nc.declare_dram_parameter("ones1", [1, 128], BF16, isOutput=False)
    qb_d = nc.declare_dram_parameter("qb_t", [128, 2], F32, isOutput=False)
    kb_d = nc.declare_dram_parameter("kb_t", [128, 2], F32, isOutput=False)
    vb_d = nc.declare_dram_parameter("vb_bc", [K, 256], F32, isOutput=False)
    fb_d = nc.declare_dram_parameter("fuse_b_bc", [128, 1], F32, isOutput=False)
    fa_d = nc.declare_dram_parameter("fuse_a_bc", [128, 1], F32, isOutput=False)
    gcna_d = nc.declare_dram_parameter("gcn_am1", [128, 4], F32, isOutput=False)
    bnb_d = nc.declare_dram_parameter("bn_b", [128, 4], F32, isOutput=False)
    outpa_d = nc.declare_dram_parameter("out_pam1", [128, 4], F32, isOutput=False)
    y_d = nc.declare_dram_parameter("y", [C, H, W], F32, isOutput=True)

    with tile.TileContext(nc) as tc, ExitStack() as ctx:
        # ---------------- persistent SBUF ----------------
        cpool = ctx.enter_context(tc.tile_pool(name="consts", bufs=1))

        def load(dram, shape, dtype=F32, tag=None):
            t = cpool.tile(shape, dtype, tag=tag, name=tag)
            nc.sync.dma_start(out=t[:], in_=dram[:])
            return t

        camw = load(camw_d, [128, 4 * K], BF16, tag="camw")
        camb = load(camb_d, [K, 1], tag="camb")
        qw = load(qw_d, [128, 1024], BF16, tag="qw")
        kw = load(kw_d, [128, 1024], BF16, tag="kw")
        vw = load(vw_d, [128, 1024], BF16, tag="vw")
        linw = load(linw_d, [128, 2048], BF16, tag="linw")
        outw8 = load(outw_d, [128, 1024], FP8, tag="outw8")
        deq = load(deq_d, [128, 1], tag="deq")
        w1s = load(w1s_d, [128, 3 * 512], BF16, tag="w1s")
        fuses = load(fuses_d, [128, 3 * K], BF16, tag="fuses")
        i128 = load(i128_d, [128, 128], BF16, tag="i128")
        si19 = load(si19_d, [128, K], BF16, tag="si19")
        si76 = load(si76_d, [128, 76], BF16, tag="si76")
        ones19 = load(ones19_d, [K, 1], BF16, tag="ones19")
        ones1 = load(ones1_d, [1, 128], BF16, tag="ones1")
        qb = load(qb_d, [128, 2], tag="qb")
        kb = load(kb_d, [128, 2], tag="kb")
        vb = load(vb_d, [K, 256], tag="vb")
        fb = load(fb_d, [128, 1], tag="fb")
        fam1 = load(fa_d, [128, 1], tag="fam1")
        gcnam1 = load(gcna_d, [128, 4], tag="gcnam1")
        bnb = load(bnb_d, [128, 4], tag="bnb")
        pam1 = load(outpa_d, [128, 4], tag="pam1")
        out_pa = load(outpa_raw_d, [128, 4], tag="outpa")

        # x quarter tiles: pool spans both passes; the last XQ_BUFS
        # quarters from pass 1 stay resident and pass 2 (reverse order)
        # reuses them for the residual instead of re-reading x
        XQ_BUFS = 8
        xq_pool = ctx.enter_context(tc.tile_pool(name="xq", bufs=XQ_BUFS))

        ppool = ctx.enter_context(tc.tile_pool(name="persist", bufs=1))
        # q in bf16, bin-major: [128 dpart, (2 dchunk, 16 bin, 1024 px)]
        q_sb = ppool.tile([128, 2 * H * W], BF16, tag="q")
        kk_sb = ppool.tile([128, 2 * 304], BF16, tag="kk")
        v_sb = ppool.tile([K, 256], BF16, tag="vsb")
        scale_v2 = ppool.tile([128, 4], F32, tag="scalev2")
        locg = [ppool.tile([114, 512], BF16, tag=f"locg{g}",
                           name=f"locg{g}") for g in range(3)]
        gstack = [ppool.tile([114, 512], BF16, tag=f"gst{g}",
                             name=f"gst{g}") for g in range(3)]

        with tc.tile_pool(name="p1acc", bufs=1) as acc_pool:
            # stacked local sums: row 32j+k = bin(4bi+j) class k, col
            # (bi, c): [128, (4 binrow, 512 c)]
            local_all = acc_pool.tile([128, 4 * C], F32, tag="localall")
            lg_bf = acc_pool.tile([128, 4 * C], BF16, tag="lgbf")
            s_parts = acc_pool.tile([K, 128], F32, tag="sparts")
            cls_parts = acc_pool.tile([K, 128], F32, tag="clsparts")
            s_tot = acc_pool.tile([K, 16], F32, tag="stot")
            cls_sig = acc_pool.tile([K, 16], F32, tag="cls")
            scale_t = acc_pool.tile([K, 16], F32, tag="scalet")
            nc.vector.memset(scale_v2[:], 0.0)
            nc.vector.memset(locg[2][:], 0.0)
            nc.vector.memset(gstack[2][:], 0.0)

            # =================== PASS 1 ===================
            xq_keep = {}
            with tc.tile_pool(name="esb", bufs=3) as e_pool, \
                 tc.tile_pool(name="xtsb", bufs=10) as xt_pool, \
                 tc.tile_pool(name="ps_xt", bufs=1, space="PSUM") as ps_xt, \
                 tc.tile_pool(name="etsb", bufs=10) as et_pool, \
                 tc.tile_pool(name="ps_cam", bufs=2, space="PSUM") as ps_cam, \
                 tc.tile_pool(name="ps_q", bufs=2, space="PSUM") as ps_q, \
                 tc.tile_pool(name="ps_et", bufs=1, space="PSUM") as ps_et, \
                 tc.tile_pool(name="ps_loc", bufs=1, space="PSUM") as ps_loc:
                # persistent PSUM pair for the block-diagonal E_T; the
                # diagonal [32,19] blocks are rewritten by transposes,
                # the off-diagonal zeros from this one-time memset
                # persist for the whole pass (both buffers share one
                # PSUM bank: allocation is bank-granular)
                pet_bank = ps_et.tile([128, 1024], BF16, tag="pet",
                                      name="pet")
                pxt_bank = ps_xt.tile([128, 2048], BF16, tag="pxt",
                                      name="pxt")
                pxt2 = [pxt_bank[:, 512 * i:512 * (i + 1)]
                        for i in range(4)]
                pet2 = [pet_bank[:, 128 * i:128 * (i + 1)]
                        for i in range(4)]
                # PSUM can't be memset in bf16; zero it with a
                # transpose whose stationary operand is all zeros
                zcol = cpool.tile([1, 128], BF16, tag="zcol", name="zcol")
                zrow = cpool.tile([1, 256], BF16, tag="zrow", name="zrow")
                nc.vector.memset(zcol[:], 0.0)
                nc.vector.memset(zrow[:], 0.0)
                for i in range(4):
                    nc.tensor.transpose(
                        pet_bank[:, 256 * i:256 * (i + 1)], zcol[:],
                        zrow[:])
                for bi in range(4):          # bin-row
                    # stacked local accumulator for this bin-row: all 32
                    # image rows (4 quarters x 8) accumulate in PSUM
                    pl = ps_loc.tile([128, 512], F32, tag="loc")
                    for qq in range(4):      # quarter (8 image rows)
                        r0 = 32 * bi + 8 * qq
                        # one batched DMA for all 4 c-chunks of the
                        # quarter (gpsimd DMA casts f32 -> bf16 in
                        # flight; fewer descriptors = less Pool time)
                        xq_t = xq_pool.tile([128, 4096], BF16, tag="xq",
                                            name="xq")
                        nc.gpsimd.dma_start(
                            out=xq_t[:].rearrange("p (c a b) -> p c a b",
                                                  c=4, a=8),
                            in_=x_d[:].rearrange("(c p) h w -> p c h w",
                                                 c=4)[:, :, r0:r0 + 8, :])
                        xq_keep[(bi, qq)] = xq_t
                        xq = [xq_t[:, 1024 * cc:1024 * (cc + 1)]
                              for cc in range(4)]

                        e_sb = e_pool.tile([K, PBIN], BF16, tag="esb")
                        e_v = e_sb[:].rearrange("p (a b) -> p a b", a=8)
                        base = 32 * bi + 2 * qq
                        # cam + exp + per-bin-slot sums (one wide exp per
                        # half; slot sums via strided 4D reduces)
                        for hh in range(2):
                            pc = ps_cam.tile([K, 512], F32, tag="cam")
                            for cc in range(4):
                                nc.tensor.matmul(
                                    pc[:], camw[:, K * cc:K * (cc + 1)],
                                    xq[cc][:, 512 * hh:512 * (hh + 1)],
                                    start=(cc == 0), stop=(cc == 3))
                            pcv = pc[:].rearrange("p (a b) -> p a b", a=4)
                            nc.scalar.activation(
                                e_v[:, 4 * hh:4 * hh + 4, :], pcv,
                                AF.Exp, bias=camb[:], scale=1.0)
                            nc.vector.reduce_sum(
                                out=cls_parts[:, base + hh:base + hh + 25:8],
                                in_=pc[:].rearrange(
                                    "p (r j w) -> p j r w", r=4, j=4),
                                axis=AX.XY)
                            nc.vector.reduce_sum(
                                out=s_parts[:, base + hh:base + hh + 25:8],
                                in_=e_sb[:].rearrange(
                                    "p (r j w) -> p j r w", r=8, j=4)[
                                    :, :, 4 * hh:4 * hh + 4, :],
                                axis=AX.XY)

                        # q projection (written bin-major) -- emitted
                        # before the local matmuls so the PE has queued
                        # work while the xts DMA transposes land
                        for dd in range(2):
                            for hh in range(2):
                                pq = ps_q.tile([128, 512], F32, tag="q")
                                for cc in range(4):
                                    nc.tensor.matmul(
                                        pq[:],
                                        qw[:, 256 * cc + 128 * dd:
                                           256 * cc + 128 * dd + 128],
                                        xq[cc][:, 512 * hh:512 * (hh + 1)],
                                        start=(cc == 0), stop=(cc == 3))
                                pqv = pq[:].rearrange(
                                    "p (r j w) -> p j r w", r=4, j=4)
                                qdst = q_sb[:].rearrange(
                                    "p (d n w) -> p d n w", d=2, n=16)[
                                    :, dd, 4 * bi:4 * bi + 4,
                                    256 * qq + 128 * hh:
                                    256 * qq + 128 * hh + 128].rearrange(
                                    "p j (r w) -> p j r w", r=4)
                                nc.scalar.activation(
                                    qdst, pqv, AF.Identity,
                                    bias=qb[:, dd:dd + 1], scale=1.0)

                        # per image row: block-diag E_T (4 tiny diagonal
                        # transposes into the persistent zero-padded
                        # PSUM tile, copied out), then the 8 local
                        # matmuls contracting 128 pixels each
                        ets = []
                        xts = []
                        for rr in range(8):  # image row within quarter
                            pet = pet2[rr % 4]
                            pxt = pxt2[rr % 4]
                            for cc in range(4):
                                nc.tensor.transpose(
                                    pxt[:, 128 * cc:128 * (cc + 1)],
                                    xq[cc][:, 128 * rr:128 * (rr + 1)],
                                    i128[:])
                            xt_sb = xt_pool.tile([128, 512], BF16,
                                                 tag="xt", name="xt_sb")
                            if rr % 2 == 0:
                                nc.scalar.copy(xt_sb[:], pxt)
                            else:
                                nc.vector.tensor_copy(xt_sb[:], pxt)
                            xts.append(xt_sb)
                            for j in range(4):
                                nc.tensor.transpose(
                                    pet[32 * j:32 * j + 32,
                                        32 * j:32 * j + K],
                                    e_sb[:, 128 * rr + 32 * j:
                                         128 * rr + 32 * j + 32],
                                    i128[:K, :K],
                                    tile_position=(0, 32 * j))
                            et_sb = et_pool.tile([128, 128], BF16,
                                                 tag="et", name="et_sb")
                            nc.vector.tensor_copy(et_sb[:], pet)
                            ets.append(et_sb)
                        for rr in range(8):
                            nc.tensor.matmul(
                                pl[:], ets[rr][:], xts[rr][:],
                                start=(qq == 0 and rr == 0),
                                stop=(qq == 3 and rr == 7))
                    nc.vector.tensor_copy(
                        local_all[:, 512 * bi:512 * (bi + 1)], pl[:])

                    # per-bin-row normalizers + local scaling + stacked
                    # GCN input layout, folded into pass 1's tail so the
                    # GCN phase starts with its matmuls immediately
                    b4 = slice(4 * bi, 4 * bi + 4)
                    nc.vector.reduce_sum(
                        out=s_tot[:, b4],
                        in_=s_parts[:, 32 * bi:32 * bi + 32].rearrange(
                            "p (n q) -> p n q", n=4),
                        axis=AX.X)
                    nc.vector.reduce_sum(
                        out=cls_sig[:, b4],
                        in_=cls_parts[:, 32 * bi:32 * bi + 32].rearrange(
                            "p (n q) -> p n q", n=4),
                        axis=AX.X)
                    nc.scalar.activation(cls_sig[:, b4], cls_sig[:, b4],
                                         AF.Sigmoid, bias=camb[:],
                                         scale=1.0 / PBIN)
                    nc.vector.reciprocal(s_tot[:, b4], s_tot[:, b4])
                    nc.vector.tensor_mul(scale_t[:, b4], cls_sig[:, b4],
                                         s_tot[:, b4])
                    # scale_v2[32j+k, bi] = scale_t[k, 4bi+j]
                    for j in range(4):
                        nc.sync.dma_start(
                            out=scale_v2[32 * j:32 * j + K, bi:bi + 1],
                            in_=scale_t[:, 4 * bi + j:4 * bi + j + 1])
                    nc.vector.tensor_scalar_mul(
                        local_all[:, 512 * bi:512 * (bi + 1)],
                        local_all[:, 512 * bi:512 * (bi + 1)],
                        scale_v2[:, bi:bi + 1])
                    nc.vector.tensor_copy(
                        lg_bf[:, 512 * bi:512 * (bi + 1)],
                        local_all[:, 512 * bi:512 * (bi + 1)])
                    for j in range(4):
                        n = 4 * bi + j
                        g, mm = n // 6, n % 6
                        nc.sync.dma_start(
                            out=locg[g][19 * mm:19 * mm + 19, :],
                            in_=lg_bf[32 * j:32 * j + K,
                                      512 * bi:512 * (bi + 1)])

            # =================== GCN ===================
            with tc.tile_pool(name="gcn", bufs=1) as gpool:
                # GCN mix into the same stacked layout; overwrites
                # local_all in place. prelu(z,a) = z + (a-1)*min(z,0)
                with tc.tile_pool(name="ps_g", bufs=2, space="PSUM") as ps_g, \
                     tc.tile_pool(name="ptmp", bufs=2) as pt_pool:
                    for bim in range(4):
                        pg = ps_g.tile([128, 512], F32, tag="g")
                        for g in range(3):
                            nc.tensor.matmul(
                                pg[:],
                                w1s[:114, 512 * g + 128 * bim:
                                    512 * g + 128 * (bim + 1)],
                                locg[g][:], start=(g == 0), stop=(g == 2))
                        z = local_all[:, 512 * bim:512 * (bim + 1)]
                        nc.vector.tensor_add(z, pg[:], z)
                        ptmp = pt_pool.tile([128, 512], F32, tag="ptmp")
                        nc.vector.tensor_scalar(
                            ptmp[:], z, 0.0, gcnam1[:, bim:bim + 1],
                            op0=ALU.min, op1=ALU.mult)
                        nc.vector.tensor_add(z, z, ptmp[:])
                        nc.vector.tensor_copy(
                            lg_bf[:, 512 * bim:512 * (bim + 1)], z)
                        for jm in range(4):
                            m = 4 * bim + jm
                            g, mm = m // 6, m % 6
                            nc.sync.dma_start(
                                out=gstack[g][19 * mm:19 * mm + 19, :],
                                in_=lg_bf[32 * jm:32 * jm + K,
                                          512 * bim:512 * (bim + 1)])

                # transpose g -> c-partition layout [128,(cchunk4, m16, k19)]
                g_ct = gpool.tile([128, 4 * 304], BF16, tag="gct")
                gf_sb = gpool.tile([K, 512], BF16, tag="gfsb")
                gf_ct = gpool.tile([128, 4 * K], BF16, tag="gfct")
                localg_ct = gpool.tile([128, 4 * 304], BF16, tag="lgct")
                glob_ct = gpool.tile([128, 4 * K], BF16, tag="glob")

                with tc.tile_pool(name="ps_t2", bufs=2, space="PSUM") as ps_t2, \
                     tc.tile_pool(name="ps_mm2", bufs=2, space="PSUM") as ps_mm2, \
                     tc.tile_pool(name="ps_sm2", bufs=2, space="PSUM") as ps_sm2:
                    # gf = sum_n fuse_w[n] g[n]  (fuse before lin: linearity)
                    pgf = ps_sm2.tile([K, 512], F32, tag="sm")
                    for g in range(3):
                        nc.tensor.matmul(pgf[:],
                                         fuses[:114, K * g:K * (g + 1)],
                                         gstack[g][:],
                                         start=(g == 0), stop=(g == 2))
                    nc.scalar.copy(gf_sb[:], pgf[:])

                    # spread stationary transposes: one [128,76] PE
                    # transpose per (bim, cc) covers all 4 bins (cols
                    # 19j+k of m=4bim+j land at g_ct col 19m)
                    for bim in range(4):
                        for cc in range(4):
                            pt = ps_t2.tile([128, 76], BF16, tag="t2")
                            nc.tensor.transpose(
                                pt[:],
                                lg_bf[:, 512 * bim + 128 * cc:
                                      512 * bim + 128 * (cc + 1)],
                                si76[:])
                            nc.scalar.copy(
                                g_ct[:, 304 * cc + 76 * bim:
                                     304 * cc + 76 * (bim + 1)], pt[:])
                    for cc in range(4):
                        pt = ps_t2.tile([128, K], BF16, tag="t2")
                        nc.tensor.transpose(
                            pt[:], gf_sb[:, 128 * cc:128 * (cc + 1)],
                            i128[:K, :K])
                        nc.scalar.copy(gf_ct[:, K * cc:K * (cc + 1)], pt[:])

                    # local_g = g @ lin_w^T : [128,(dchunk,m,k)]
                    for ddc in range(4):
                        plg = ps_mm2.tile([128, 304], F32, tag="mm2")
                        for cc in range(4):
                            nc.tensor.matmul(
                                plg[:],
                                linw[:, 512 * cc + 128 * ddc:
                                     512 * cc + 128 * ddc + 128],
                                g_ct[:, 304 * cc:304 * (cc + 1)],
                                start=(cc == 0), stop=(cc == 3))
                        nc.scalar.copy(localg_ct[:, 304 * ddc:304 * (ddc + 1)],
                                       plg[:])

                    # kk = local_g @ k_w^T + k_b -> bf16 [128,(di2, m, k)]
                    for di in range(2):
                        pkk = ps_mm2.tile([128, 304], F32, tag="mm2")
                        for cc in range(4):
                            nc.tensor.matmul(
                                pkk[:],
                                kw[:, 256 * cc + 128 * di:
                                   256 * cc + 128 * di + 128],
                                localg_ct[:, 304 * cc:304 * (cc + 1)],
                                start=(cc == 0), stop=(cc == 3))
                        nc.scalar.activation(
                            kk_sb[:, 304 * di:304 * (di + 1)], pkk[:],
                            AF.Identity, bias=kb[:, di:di + 1], scale=1.0)

                    # glob = prelu(gf @ lin_w^T + fuse_b) -> [128,(cchunk4,k)]
                    for ddc in range(4):
                        pgl = ps_sm2.tile([128, K], F32, tag="smg")
                        for cc in range(4):
                            nc.tensor.matmul(
                                pgl[:],
                                linw[:, 512 * cc + 128 * ddc:
                                     512 * cc + 128 * ddc + 128],
                                gf_ct[:, K * cc:K * (cc + 1)],
                                start=(cc == 0), stop=(cc == 3))
                        gz = glob_ct[:, K * ddc:K * (ddc + 1)]
                        nc.scalar.activation(gz, pgl[:], AF.Identity,
                                             bias=fb[:], scale=1.0)
                        gtmp = gpool.tile([128, K], BF16, tag="gtmp",
                                          name=f"gtmp{ddc}")
                        nc.vector.tensor_scalar(
                            gtmp[:], gz, 0.0, fam1[:],
                            op0=ALU.min, op1=ALU.mult)
                        nc.vector.tensor_add(gz, gz, gtmp[:])

                    # v = glob @ v_w^T + v_b : [19, 256] bf16
                    pv = ps_sm2.tile([K, 512], F32, tag="sm")
                    for cc in range(4):
                        nc.tensor.matmul(
                            pv[:, :256], glob_ct[:, K * cc:K * (cc + 1)],
                            vw[:, 256 * cc:256 * (cc + 1)],
                            start=(cc == 0), stop=(cc == 3))
                    nc.vector.tensor_add(v_sb[:], pv[:, :256], vb[:])

        # =================== PASS 2 ===================
        tc.strict_bb_all_engine_barrier()
        q_v = q_sb[:].rearrange("p (d n w) -> p d n w", d=2, n=16)
        with tc.tile_pool(name="osb", bufs=2) as o_pool, \
             tc.tile_pool(name="eaff", bufs=2) as ea_pool, \
             tc.tile_pool(name="ssb", bufs=2) as s_pool, \
             tc.tile_pool(name="sinvb", bufs=2) as si_pool, \
             tc.tile_pool(name="xr", bufs=3) as xr_pool, \
             tc.tile_pool(name="wsb", bufs=3) as w_pool, \
             tc.tile_pool(name="ps_aff", bufs=2, space="PSUM") as ps_aff, \
             tc.tile_pool(name="ps_sp", bufs=1, space="PSUM") as ps_sp, \
             tc.tile_pool(name="ps_sb", bufs=1, space="PSUM") as ps_sb, \
             tc.tile_pool(name="ps_o", bufs=2, space="PSUM") as ps_o, \
             tc.tile_pool(name="ps_y", bufs=2, space="PSUM") as ps_y:
            for bi in range(4):
                # x rows for the residual: quarters still resident from
                # pass 1 are reused in place; older ones are prefetched
                # (bf16, cast in flight, one batched DMA per quarter)
                xr_tiles = {}
                for qq in range(4):
                    if 4 * bi + qq >= 16 - XQ_BUFS:
                        xr_tiles[qq] = xq_keep[(bi, qq)]
                        continue
                    r0 = 32 * bi + 8 * qq
                    xr = xr_pool.tile([128, 4096], BF16, tag="xr",
                                      name="xr")
                    nc.gpsimd.dma_start(
                        out=xr[:].rearrange("p (c a b) -> p c a b",
                                            c=4, a=8),
                        in_=x_d[:].rearrange("(c p) h w -> p c h w",
                                             c=4)[:, :, r0:r0 + 8, :])
                    xr_tiles[qq] = xr
                # --- 2A: attention per bin ---
                # o stored fp8 (x16 scale folded into v_w/v_b on the
                # host), contraction-pair interleaved for the DoubleRow
                # out-conv: element (j, px, di) at col 2048j + 2px + di
                o_sb = o_pool.tile([128, 2 * 4 * PBIN], FP8, tag="osb")
                for j in range(4):
                    n = 4 * bi + j
                    eaff = ea_pool.tile([K, PBIN], BF16, tag="eaff")
                    s_sb = s_pool.tile([1, PBIN], BF16, tag="ssb")
                    for hh in range(2):
                        sinv = si_pool.tile([128, 512], F32, tag="sinvb")
                        pa = ps_aff.tile([K, 512], F32, tag="aff")
                        for di in range(2):
                            nc.tensor.matmul(
                                pa[:],
                                kk_sb[:, 304 * di + K * n:
                                      304 * di + K * (n + 1)],
                                q_v[:, di, n, 512 * hh:512 * (hh + 1)],
                                start=(di == 0), stop=(di == 1))
                        nc.scalar.activation(
                            eaff[:, 512 * hh:512 * (hh + 1)], pa[:],
                            AF.Exp, bias=0.0, scale=1.0)
                        psx = ps_sp.tile([1, 512], F32, tag="sp")
                        nc.tensor.matmul(psx[:], ones19[:],
                                         eaff[:, 512 * hh:512 * (hh + 1)],
                                         start=True, stop=True)
                        nc.scalar.copy(s_sb[:, 512 * hh:512 * (hh + 1)],
                                       psx[:])
                        pb = ps_sb.tile([128, 512], F32, tag="sb")
                        nc.tensor.matmul(pb[:], ones1[:],
                                         s_sb[:, 512 * hh:512 * (hh + 1)],
                                         start=True, stop=True)
                        nc.vector.reciprocal(sinv[:], pb[:])
                        for di in range(2):
                            po = ps_o.tile([128, 512], F32, tag="o")
                            nc.tensor.matmul(
                                po[:], v_sb[:, 128 * di:128 * (di + 1)],
                                eaff[:, 512 * hh:512 * (hh + 1)],
                                start=True, stop=True)
                            nc.vector.tensor_mul(
                                o_sb[:, 2048 * j + 1024 * hh + di:
                                     2048 * j + 1024 * hh + di + 1023:2],
                                po[:], sinv[:])
                # --- 2B: out conv + BN + prelu + residual per quarter-row --
                # bn scale is folded into out_wT on the host; here:
                # w = prelu(conv + bn_b, a) on ACT (in-place on PSUM),
                # then one DVE add for the residual
                for qq in range(4):
                    r0 = 32 * bi + 8 * qq
                    xr_t = xr_tiles[qq]
                    for cc in range(4):
                        xrv = xr_t[:, 1024 * cc:1024 * (cc + 1)].rearrange(
                            "p (a b) -> p a b", a=8)
                        for jp in range(2):      # bin pair (j0, j0+1)
                            j0 = 2 * jp
                            py = ps_y.tile([128, 512], F32, tag="y")
                            for dj in range(2):
                                j = j0 + dj
                                nc.tensor.matmul(
                                    py[:, 256 * dj:256 * (dj + 1)],
                                    outw8[:].rearrange(
                                        "p (i c) -> p i c", i=2)[
                                        :, :, 128 * cc:128 * (cc + 1)],
                                    o_sb[:, 2048 * j + 512 * qq:
                                         2048 * j + 512 * qq +
                                         512].rearrange(
                                        "p (w i) -> p i w", i=2),
                                    perf_mode=mybir.MatmulPerfMode.DoubleRow,
                                    start=True, stop=True)
                            w_sb = w_pool.tile([128, 512], BF16, tag="w",
                                               name="w_sb")
                            # deq = 1/(sw*so) dequantizes the fp8 conv
                            nc.scalar.activation(
                                w_sb[:], py[:], AF.Prelu,
                                bias=bnb[:, cc:cc + 1], scale=deq[:],
                                alpha=out_pa[:, cc:cc + 1])
                            # w free layout (j2, r8, w32) -> xr (r8, j2*w32)
                            wv = w_sb[:].rearrange(
                                "p (j r w) -> p r j w", j=2, r=8)
                            xrj = xrv[:, :, 32 * j0:32 * j0 + 64].rearrange(
                                "p r (j w) -> p r j w", j=2)
                            nc.vector.tensor_add(xrj, wv, xrj)
                    # one batched y write per quarter (casts bf16 -> f32)
                    nc.gpsimd.dma_start(
                        out=y_d[:].rearrange("(c p) h w -> p c h w",
                                             c=4)[:, :, r0:r0 + 8, :],
                        in_=xr_t[:].rearrange("p (c a b) -> p c a b",
                                              c=4, a=8))
    return nc


def split_excess_waits(nc, max_waits=1):
    """Walrus rejects instructions with more than `max_waits` sync-wait
    commands. Move excess waits onto preceding same-engine NoOps (engine
    queues are in-order, so this is semantics-preserving)."""
    n_split = 0
    for f in nc.m.functions:
        for blk in f.blocks:
            new = []
            for inst in blk.instructions:
                si = inst.sync_info
                if si is not None and si.on_wait and len(si.on_wait) > max_waits:
                    waits = list(si.on_wait)
                    k = 0
                    while len(waits) > max_waits:
                        chunk, waits = waits[:max_waits], waits[max_waits:]
                        nop = mybir.InstNoOp(
                            name=f"{inst.name}-ws{k}",
                            engine=inst.engine,
                            sync_info=mybir.SyncInfo(on_wait=chunk,
                                                     on_update=[]),
                            bass_nofuse=True,
                        )
                        new.append(nop)
                        k += 1
                        n_split += 1
                    inst.sync_info = mybir.SyncInfo(
                        on_wait=waits, on_update=list(si.on_update))
                new.append(inst)
            blk.instructions[:] = new
    return n_split


_NC_CACHE = {}


def get_nc():
    if "nc" not in _NC_CACHE:
        nc = build_nc()
        split_excess_waits(nc)
        _NC_CACHE["nc"] = nc
    return _NC_CACHE["nc"]


def prep_inputs(inputs):
    """Host-side re-layout of the module parameters (per-core, shared)."""
    f = lambda a: np.asarray(a, dtype=np.float32)
    bf = ml_dtypes.bfloat16
    conv_cam_w = f(inputs["conv_cam_w"])
    q_w, k_w, v_w = f(inputs["q_w"]), f(inputs["k_w"]), f(inputs["v_w"])
    lin_w = f(inputs["gcn_lin_w"])
    out_w = f(inputs["out_conv_w"])
    w1 = f(inputs["gcn_conv1_w"])
    fuse_w = f(inputs["fuse_w"])

    def chunkT(w, nchunk):  # [D, C] -> [128, (cchunk, D)]
        D = w.shape[0]
        return np.ascontiguousarray(
            w.T.reshape(nchunk, 128, D).transpose(1, 0, 2).reshape(
                128, nchunk * D))

    # w1s[19nn+i, 512g + 32jm + k] = W1[4bim+jm, 6g+nn] * (i==k), per bim
    w1s = np.zeros((128, 3, 4, 128), np.float32)
    fuse_s = np.zeros((128, 3 * K), np.float32)
    eye19 = np.eye(K, dtype=np.float32)
    for n in range(NBINS):
        g, nn = n // 6, n % 6
        for m in range(NBINS):
            bim, jm = m // 4, m % 4
            w1s[19 * nn:19 * nn + 19, g, bim,
                32 * jm:32 * jm + 19] = eye19 * w1[m, n]
        fuse_s[19 * nn:19 * nn + 19, K * g:K * (g + 1)] = eye19 * fuse_w[n]
    w1s = w1s.reshape(128, 3 * 512)

    # si19[32j + i, k] = (i == k) stacked identity
    si19 = np.zeros((128, K), np.float32)
    for j in range(4):
        si19[32 * j:32 * j + 19, :] = eye19
    # si76[32j + i, 19j + k] = (i == k): spread stacked identity
    si76 = np.zeros((128, 76), np.float32)
    for j in range(4):
        si76[32 * j:32 * j + 19, 19 * j:19 * j + 19] = eye19

    # gcn prelu alphas in stacked layout: row 32j+k, col bim -> a[4bim+j]-1
    gcn_am1 = np.zeros((128, 4), np.float32)
    ga = f(inputs["gcn_prelu_a"]) - 1.0
    for bim in range(4):
        for jm in range(4):
            gcn_am1[32 * jm:32 * jm + 32, bim] = ga[4 * bim + jm]

    inv = 1.0 / np.sqrt(f(inputs["bn_var"]) + 1e-5)
    bn_a = f(inputs["bn_gamma"]) * inv
    bn_b = f(inputs["bn_beta"]) - f(inputs["bn_mean"]) * bn_a
    out_w_bn = bn_a[:, None] * out_w  # fold BN scale into the conv weights

    # fp8 out-conv: weights quantized with scale sw, o with so (folded
    # into v_w/v_b); the Prelu input scale dequantizes by 1/(sw*so)
    SO = 16.0
    sw = float(2.0 ** np.floor(np.log2(224.0 / max(np.abs(out_w_bn).max(),
                                                   1e-30))))
    out_w8 = np.zeros((128, 1024), np.float32)
    for i in range(2):
        # out_w8[p, 512i + cout] = out_w_bn[cout, p + 128i] * sw
        out_w8[:, 512 * i:512 * (i + 1)] = (out_w_bn[:, 128 * i:128 * (i + 1)]
                                            * sw).T

    return {
        "cam_wT": chunkT(conv_cam_w, 4).astype(bf),
        "cam_b": f(inputs["conv_cam_b"]).reshape(K, 1),
        "q_wT": chunkT(q_w, 4).astype(bf),
        "k_wT": chunkT(k_w, 4).astype(bf),
        "v_wT": chunkT(v_w * SO, 4).astype(bf),
        "lin_wT": chunkT(lin_w, 4).astype(bf),
        "out_w8": out_w8.astype(ml_dtypes.float8_e4m3),
        "out_deq": np.full((128, 1), 1.0 / (sw * SO), np.float32),
        "w1s": w1s.astype(bf),
        "fuse_s": fuse_s.astype(bf),
        "i128": np.eye(128, dtype=np.float32).astype(bf),
        "si19": si19.astype(bf),
        "si76": si76.astype(bf),
        "ones19": np.ones((K, 1), bf),
        "ones1": np.ones((1, 128), bf),
        "qb_t": np.ascontiguousarray(f(inputs["q_b"]).reshape(2, 128).T),
        "kb_t": np.ascontiguousarray(f(inputs["k_b"]).reshape(2, 128).T),
        "vb_bc": np.tile(f(inputs["v_b"])[None, :] * SO, (K, 1)),
        "fuse_b_bc": np.full((128, 1), f(inputs["fuse_b"])[0], np.float32),
        "fuse_a_bc": np.full(
            (128, 1), f(inputs["fuse_prelu_a"])[0] - 1.0, np.float32),
        "gcn_am1": gcn_am1,
        "bn_b": np.ascontiguousarray(bn_b.reshape(4, 128).T),
        "out_pam1": np.ascontiguousarray(
            (f(inputs["out_prelu_a"]) - 1.0).reshape(4, 128).T),
        "out_pa": np.ascontiguousarray(
            f(inputs["out_prelu_a"]).reshape(4, 128).T),
    }


def kernel(**inputs):
    from concourse.bass_utils import run_bass_kernel_spmd
    nc = get_nc()
    params = prep_inputs(inputs)
    x = np.asarray(inputs["x"], dtype=np.float32)
    in_maps = [dict(params, x=np.ascontiguousarray(x[b]))
               for b in range(NCORES)]
    res = run_bass_kernel_spmd(nc, in_maps, list(range(NCORES)))
    return np.stack([res.results[b]["y"] for b in range(NCORES)], axis=0)



# revision 78
# speedup vs baseline: 1.0812x; 1.0812x over previous
"""Trainium2 Bass kernel for the CAAM sparse-attention module.

Data-parallel over batch B=8 across 8 NeuronCores (one image per core).
All parameters replicated. Matmul fabric runs in bf16 (fp32 PSUM
accumulation); the out-conv runs in fp8e4m3 with the DoubleRow perf
mode (per-tensor power-of-2 scales folded into v_w/v_b host-side and
dequantized by the Prelu input scale); softmax normalizers and biases
stay fp32; the residual path is bf16 (x re-read cast in flight, y
written through a bf16->f32 cast DMA).

Pass 1 streams x once as [128c-chunk, (4cc, 8 rows, 128 cols)] bf16
quarter tiles (one batched gpsimd cast-DMA each). Per image row the
per-bin pixel contraction (local = pixconf @ x_p) does 4 tiny diagonal
transposes of exp(cam) into a persistent zero-padded block-diagonal
PSUM stationary, then ONE matmul contracting all 128 pixels, with the
whole bin-row (32 rows) accumulating in a single PSUM bank. Per-bin-row
softmax/sigmoid normalizers, local scaling and the stacked GCN input
layout are folded into the pass-1 tail. BN+PReLU+residual is one ACT
Prelu (per-partition alpha) plus one DVE add per tile. Pass 2 reuses
the last 8 resident x quarter tiles for the residual (bin-rows 2-3)
and prefetches the rest.
"""

import os

os.environ.setdefault("JAX_COMPILATION_CACHE_DIR", "/tmp/jax_comp_cache")
os.environ.setdefault("MYCRO_LOCAL_CACHE", "1")

import numpy as np
import ml_dtypes

import concourse.bass as bass
import concourse.mybir as mybir
import concourse.tile as tile
from contextlib import ExitStack

dt = mybir.dt
F32 = dt.float32
BF16 = dt.bfloat16
FP8 = dt.float8e4
AX = mybir.AxisListType
AF = mybir.ActivationFunctionType
ALU = mybir.AluOpType

C, H, W, K, CI = 512, 128, 128, 19, 256
NBINS = 16          # 4x4 bins
PBIN = 1024         # 32*32 pixels per bin
NCORES = 8


def build_nc():
    nc = bass.Bass("TRN2", target_bir_lowering=False, debug=False)

    x_d = nc.declare_dram_parameter("x", [C, H, W], F32, isOutput=False)
    camw_d = nc.declare_dram_parameter("cam_wT", [128, 4 * K], BF16, isOutput=False)
    camb_d = nc.declare_dram_parameter("cam_b", [K, 1], F32, isOutput=False)
    qw_d = nc.declare_dram_parameter("q_wT", [128, 1024], BF16, isOutput=False)
    kw_d = nc.declare_dram_parameter("k_wT", [128, 1024], BF16, isOutput=False)
    vw_d = nc.declare_dram_parameter("v_wT", [128, 1024], BF16, isOutput=False)
    linw_d = nc.declare_dram_parameter("lin_wT", [128, 2048], BF16, isOutput=False)
    outw_d = nc.declare_dram_parameter("out_w8", [128, 1024], FP8, isOutput=False)
    deq_d = nc.declare_dram_parameter("out_deq", [128, 1], F32, isOutput=False)
    w1s_d = nc.declare_dram_parameter("w1s", [128, 3 * 512], BF16, isOutput=False)
    fuses_d = nc.declare_dram_parameter("fuse_s", [128, 3 * K], BF16, isOutput=False)
    i128_d = nc.declare_dram_parameter("i128", [128, 128], BF16, isOutput=False)
    si19_d = nc.declare_dram_parameter("si19", [128, K], BF16, isOutput=False)
    si76_d = nc.declare_dram_parameter("si76", [128, 76], BF16, isOutput=False)
    outpa_raw_d = nc.declare_dram_parameter("out_pa", [128, 4], F32, isOutput=False)
    ones19_d = nc.declare_dram_parameter("ones19", [K, 1], BF16, isOutput=False)
    ones1_d = nc.declare_dram_parameter("ones1", [1, 128], BF16, isOutput=False)
    qb_d = nc.declare_dram_parameter("qb_t", [128, 2], F32, isOutput=False)
    kb_d = nc.declare_dram_parameter("kb_t", [128, 2], F32, isOutput=False)
    vb_d = nc.declare_dram_parameter("vb_bc", [K, 256], F32, isOutput=False)
    fb_d = nc.declare_dram_parameter("fuse_b_bc", [128, 1], F32, isOutput=False)
    fa_d = nc.declare_dram_parameter("fuse_a_bc", [128, 1], F32, isOutput=False)
    gcna_d = nc.declare_dram_parameter("gcn_am1", [128, 4], F32, isOutput=False)
    bnb_d = nc.declare_dram_parameter("bn_b", [128, 4], F32, isOutput=False)
    outpa_d = nc.declare_dram_parameter("out_pam1", [128, 4], F32, isOutput=False)
    y_d = nc.declare_dram_parameter("y", [C, H, W], F32, isOutput=True)

    with tile.TileContext(nc) as tc, ExitStack() as ctx:
        # ---------------- persistent SBUF ----------------
        cpool = ctx.enter_context(tc.tile_pool(name="consts", bufs=1))

        def load(dram, shape, dtype=F32, tag=None):
            t = cpool.tile(shape, dtype, tag=tag, name=tag)
            nc.sync.dma_start(out=t[:], in_=dram[:])
            return t

        camw = load(camw_d, [128, 4 * K], BF16, tag="camw")
        camb = load(camb_d, [K, 1], tag="camb")
        qw = load(qw_d, [128, 1024], BF16, tag="qw")
        kw = load(kw_d, [128, 1024], BF16, tag="kw")
        vw = load(vw_d, [128, 1024], BF16, tag="vw")
        linw = load(linw_d, [128, 2048], BF16, tag="linw")
        outw8 = load(outw_d, [128, 1024], FP8, tag="outw8")
        deq = load(deq_d, [128, 1], tag="deq")
        w1s = load(w1s_d, [128, 3 * 512], BF16, tag="w1s")
        fuses = load(fuses_d, [128, 3 * K], BF16, tag="fuses")
        i128 = load(i128_d, [128, 128], BF16, tag="i128")
        si19 = load(si19_d, [128, K], BF16, tag="si19")
        si76 = load(si76_d, [128, 76], BF16, tag="si76")
        ones19 = load(ones19_d, [K, 1], BF16, tag="ones19")
        ones1 = load(ones1_d, [1, 128], BF16, tag="ones1")
        qb = load(qb_d, [128, 2], tag="qb")
        kb = load(kb_d, [128, 2], tag="kb")
        vb = load(vb_d, [K, 256], tag="vb")
        fb = load(fb_d, [128, 1], tag="fb")
        fam1 = load(fa_d, [128, 1], tag="fam1")
        gcnam1 = load(gcna_d, [128, 4], tag="gcnam1")
        bnb = load(bnb_d, [128, 4], tag="bnb")
        pam1 = load(outpa_d, [128, 4], tag="pam1")
        out_pa = load(outpa_raw_d, [128, 4], tag="outpa")

        # x quarter tiles: pool spans both passes; the last XQ_BUFS
        # quarters from pass 1 stay resident and pass 2 (reverse order)
        # reuses them for the residual instead of re-reading x
        XQ_BUFS = 8
        xq_pool = ctx.enter_context(tc.tile_pool(name="xq", bufs=XQ_BUFS))

        ppool = ctx.enter_context(tc.tile_pool(name="persist", bufs=1))
        # q in bf16, bin-major: [128 dpart, (2 dchunk, 16 bin, 1024 px)]
        q_sb = ppool.tile([128, 2 * H * W], BF16, tag="q")
        kk_sb = ppool.tile([128, 2 * 304], BF16, tag="kk")
        v_sb = ppool.tile([K, 256], BF16, tag="vsb")
        scale_v2 = ppool.tile([128, 4], F32, tag="scalev2")
        locg = [ppool.tile([114, 512], BF16, tag=f"locg{g}",
                           name=f"locg{g}") for g in range(3)]
        gstack = [ppool.tile([114, 512], BF16, tag=f"gst{g}",
                             name=f"gst{g}") for g in range(3)]

        with tc.tile_pool(name="p1acc", bufs=1) as acc_pool:
            # stacked local sums: row 32j+k = bin(4bi+j) class k, col
            # (bi, c): [128, (4 binrow, 512 c)]
            local_all = acc_pool.tile([128, 4 * C], F32, tag="localall")
            lg_bf = acc_pool.tile([128, 4 * C], BF16, tag="lgbf")
            s_parts = acc_pool.tile([K, 128], F32, tag="sparts")
            cls_parts = acc_pool.tile([K, 128], F32, tag="clsparts")
            s_tot = acc_pool.tile([K, 16], F32, tag="stot")
            cls_sig = acc_pool.tile([K, 16], F32, tag="cls")
            scale_t = acc_pool.tile([K, 16], F32, tag="scalet")
            nc.vector.memset(scale_v2[:], 0.0)
            nc.vector.memset(locg[2][:], 0.0)
            nc.vector.memset(gstack[2][:], 0.0)

            # =================== PASS 1 ===================
            xq_keep = {}
            with tc.tile_pool(name="esb", bufs=3) as e_pool, \
                 tc.tile_pool(name="xtsb", bufs=10) as xt_pool, \
                 tc.tile_pool(name="ps_xt", bufs=1, space="PSUM") as ps_xt, \
                 tc.tile_pool(name="etsb", bufs=10) as et_pool, \
                 tc.tile_pool(name="ps_cam", bufs=2, space="PSUM") as ps_cam, \
                 tc.tile_pool(name="ps_q", bufs=2, space="PSUM") as ps_q, \
                 tc.tile_pool(name="ps_et", bufs=1, space="PSUM") as ps_et, \
                 tc.tile_pool(name="ps_loc", bufs=1, space="PSUM") as ps_loc:
                # persistent PSUM pair for the block-diagonal E_T; the
                # diagonal [32,19] blocks are rewritten by transposes,
                # the off-diagonal zeros from this one-time memset
                # persist for the whole pass (both buffers share one
                # PSUM bank: allocation is bank-granular)
                pet_bank = ps_et.tile([128, 1024], BF16, tag="pet",
                                      name="pet")
                pxt_bank = ps_xt.tile([128, 2048], BF16, tag="pxt",
                                      name="pxt")
                pxt2 = [pxt_bank[:, 512 * i:512 * (i + 1)]
                        for i in range(4)]
                pet2 = [pet_bank[:, 128 * i:128 * (i + 1)]
                        for i in range(4)]
                # PSUM can't be memset in bf16; zero it with a
                # transpose whose stationary operand is all zeros
                zcol = cpool.tile([1, 128], BF16, tag="zcol", name="zcol")
                zrow = cpool.tile([1, 256], BF16, tag="zrow", name="zrow")
                nc.vector.memset(zcol[:], 0.0)
                nc.vector.memset(zrow[:], 0.0)
                for i in range(4):
                    nc.tensor.transpose(
                        pet_bank[:, 256 * i:256 * (i + 1)], zcol[:],
                        zrow[:])
                for bi in range(4):          # bin-row
                    # stacked local accumulator for this bin-row: all 32
                    # image rows (4 quarters x 8) accumulate in PSUM
                    pl = ps_loc.tile([128, 512], F32, tag="loc")
                    for qq in range(4):      # quarter (8 image rows)
                        r0 = 32 * bi + 8 * qq
                        # one batched DMA for all 4 c-chunks of the
                        # quarter (gpsimd DMA casts f32 -> bf16 in
                        # flight; fewer descriptors = less Pool time)
                        xq_t = xq_pool.tile([128, 4096], BF16, tag="xq",
                                            name="xq")
                        nc.gpsimd.dma_start(
                            out=xq_t[:].rearrange("p (c a b) -> p c a b",
                                                  c=4, a=8),
                            in_=x_d[:].rearrange("(c p) h w -> p c h w",
                                                 c=4)[:, :, r0:r0 + 8, :])
                        xq_keep[(bi, qq)] = xq_t
                        xq = [xq_t[:, 1024 * cc:1024 * (cc + 1)]
                              for cc in range(4)]

                        e_sb = e_pool.tile([K, PBIN], BF16, tag="esb")
                        e_v = e_sb[:].rearrange("p (a b) -> p a b", a=8)
                        base = 32 * bi + 2 * qq
                        # cam + exp + per-bin-slot sums (one wide exp per
                        # half; slot sums via strided 4D reduces)
                        for hh in range(2):
                            pc = ps_cam.tile([K, 512], F32, tag="cam")
                            for cc in range(4):
                                nc.tensor.matmul(
                                    pc[:], camw[:, K * cc:K * (cc + 1)],
                                    xq[cc][:, 512 * hh:512 * (hh + 1)],
                                    start=(cc == 0), stop=(cc == 3))
                            pcv = pc[:].rearrange("p (a b) -> p a b", a=4)
                            nc.scalar.activation(
                                e_v[:, 4 * hh:4 * hh + 4, :], pcv,
                                AF.Exp, bias=camb[:], scale=1.0)
                            nc.vector.reduce_sum(
                                out=cls_parts[:, base + hh:base + hh + 25:8],
                                in_=pc[:].rearrange(
                                    "p (r j w) -> p j r w", r=4, j=4),
                                axis=AX.XY)
                            nc.vector.reduce_sum(
                                out=s_parts[:, base + hh:base + hh + 25:8],
                                in_=e_sb[:].rearrange(
                                    "p (r j w) -> p j r w", r=8, j=4)[
                                    :, :, 4 * hh:4 * hh + 4, :],
                                axis=AX.XY)

                        # q projection (written bin-major) -- emitted
                        # before the local matmuls so the PE has queued
                        # work while the xts DMA transposes land
                        for dd in range(2):
                            for hh in range(2):
                                pq = ps_q.tile([128, 512], F32, tag="q")
                                for cc in range(4):
                                    nc.tensor.matmul(
                                        pq[:],
                                        qw[:, 256 * cc + 128 * dd:
                                           256 * cc + 128 * dd + 128],
                                        xq[cc][:, 512 * hh:512 * (hh + 1)],
                                        start=(cc == 0), stop=(cc == 3))
                                pqv = pq[:].rearrange(
                                    "p (r j w) -> p j r w", r=4, j=4)
                                qdst = q_sb[:].rearrange(
                                    "p (d n w) -> p d n w", d=2, n=16)[
                                    :, dd, 4 * bi:4 * bi + 4,
                                    256 * qq + 128 * hh:
                                    256 * qq + 128 * hh + 128].rearrange(
                                    "p j (r w) -> p j r w", r=4)
                                nc.scalar.activation(
                                    qdst, pqv, AF.Identity,
                                    bias=qb[:, dd:dd + 1], scale=1.0)

                        # per image row: block-diag E_T (4 tiny diagonal
                        # transposes into the persistent zero-padded
                        # PSUM tile, copied out), then the 8 local
                        # matmuls contracting 128 pixels each
                        ets = []
                        xts = []
                        for rr in range(8):  # image row within quarter
                            pet = pet2[rr % 4]
                            pxt = pxt2[rr % 4]
                            for cc in range(4):
                                nc.tensor.transpose(
                                    pxt[:, 128 * cc:128 * (cc + 1)],
                                    xq[cc][:, 128 * rr:128 * (rr + 1)],
                                    i128[:])
                            xt_sb = xt_pool.tile([128, 512], BF16,
                                                 tag="xt", name="xt_sb")
                            if rr % 2 == 0:
                                nc.scalar.copy(xt_sb[:], pxt)
                            else:
                                nc.vector.tensor_copy(xt_sb[:], pxt)
                            xts.append(xt_sb)
                            for j in range(4):
                                nc.tensor.transpose(
                                    pet[32 * j:32 * j + 32,
                                        32 * j:32 * j + K],
                                    e_sb[:, 128 * rr + 32 * j:
                                         128 * rr + 32 * j + 32],
                                    i128[:K, :K],
                                    tile_position=(0, 32 * j))
                            et_sb = et_pool.tile([128, 128], BF16,
                                                 tag="et", name="et_sb")
                            nc.vector.tensor_copy(et_sb[:], pet)
                            ets.append(et_sb)
                        for rr in range(8):
                            nc.tensor.matmul(
                                pl[:], ets[rr][:], xts[rr][:],
                                start=(qq == 0 and rr == 0),
                                stop=(qq == 3 and rr == 7))
                    nc.vector.tensor_copy(
                        local_all[:, 512 * bi:512 * (bi + 1)], pl[:])

                    # per-bin-row normalizers + local scaling + stacked
                    # GCN input layout, folded into pass 1's tail so the
                    # GCN phase starts with its matmuls immediately
                    b4 = slice(4 * bi, 4 * bi + 4)
                    nc.vector.reduce_sum(
                        out=s_tot[:, b4],
                        in_=s_parts[:, 32 * bi:32 * bi + 32].rearrange(
                            "p (n q) -> p n q", n=4),
                        axis=AX.X)
                    nc.vector.reduce_sum(
                        out=cls_sig[:, b4],
                        in_=cls_parts[:, 32 * bi:32 * bi + 32].rearrange(
                            "p (n q) -> p n q", n=4),
                        axis=AX.X)
                    nc.scalar.activation(cls_sig[:, b4], cls_sig[:, b4],
                                         AF.Sigmoid, bias=camb[:],
                                         scale=1.0 / PBIN)
                    nc.vector.reciprocal(s_tot[:, b4], s_tot[:, b4])
                    nc.vector.tensor_mul(scale_t[:, b4], cls_sig[:, b4],
                                         s_tot[:, b4])
                    # scale_v2[32j+k, bi] = scale_t[k, 4bi+j]
                    for j in range(4):
                        nc.sync.dma_start(
                            out=scale_v2[32 * j:32 * j + K, bi:bi + 1],
                            in_=scale_t[:, 4 * bi + j:4 * bi + j + 1])
                    nc.vector.tensor_scalar_mul(
                        local_all[:, 512 * bi:512 * (bi + 1)],
                        local_all[:, 512 * bi:512 * (bi + 1)],
                        scale_v2[:, bi:bi + 1])
                    nc.vector.tensor_copy(
                        lg_bf[:, 512 * bi:512 * (bi + 1)],
                        local_all[:, 512 * bi:512 * (bi + 1)])
                    for j in range(4):
                        n = 4 * bi + j
                        g, mm = n // 6, n % 6
                        nc.sync.dma_start(
                            out=locg[g][19 * mm:19 * mm + 19, :],
                            in_=lg_bf[32 * j:32 * j + K,
                                      512 * bi:512 * (bi + 1)])

            # =================== GCN ===================
            with tc.tile_pool(name="gcn", bufs=1) as gpool:
                # GCN mix into the same stacked layout; overwrites
                # local_all in place. prelu(z,a) = z + (a-1)*min(z,0)
                with tc.tile_pool(name="ps_g", bufs=2, space="PSUM") as ps_g, \
                     tc.tile_pool(name="ptmp", bufs=2) as pt_pool:
                    for bim in range(4):
                        pg = ps_g.tile([128, 512], F32, tag="g")
                        for g in range(3):
                            nc.tensor.matmul(
                                pg[:],
                                w1s[:114, 512 * g + 128 * bim:
                                    512 * g + 128 * (bim + 1)],
                                locg[g][:], start=(g == 0), stop=(g == 2))
                        z = local_all[:, 512 * bim:512 * (bim + 1)]
                        nc.vector.tensor_add(z, pg[:], z)
                        ptmp = pt_pool.tile([128, 512], F32, tag="ptmp")
                        nc.vector.tensor_scalar(
                            ptmp[:], z, 0.0, gcnam1[:, bim:bim + 1],
                            op0=ALU.min, op1=ALU.mult)
                        nc.vector.tensor_add(z, z, ptmp[:])
                        nc.vector.tensor_copy(
                            lg_bf[:, 512 * bim:512 * (bim + 1)], z)
                        for jm in range(4):
                            m = 4 * bim + jm
                            g, mm = m // 6, m % 6
                            nc.sync.dma_start(
                                out=gstack[g][19 * mm:19 * mm + 19, :],
                                in_=lg_bf[32 * jm:32 * jm + K,
                                          512 * bim:512 * (bim + 1)])

                # transpose g -> c-partition layout [128,(cchunk4, m16, k19)]
                g_ct = gpool.tile([128, 4 * 304], BF16, tag="gct")
                gf_sb = gpool.tile([K, 512], BF16, tag="gfsb")
                gf_ct = gpool.tile([128, 4 * K], BF16, tag="gfct")
                localg_ct = gpool.tile([128, 4 * 304], BF16, tag="lgct")
                glob_ct = gpool.tile([128, 4 * K], BF16, tag="glob")

                with tc.tile_pool(name="ps_t2", bufs=2, space="PSUM") as ps_t2, \
                     tc.tile_pool(name="ps_mm2", bufs=2, space="PSUM") as ps_mm2, \
                     tc.tile_pool(name="ps_sm2", bufs=2, space="PSUM") as ps_sm2:
                    # gf = sum_n fuse_w[n] g[n]  (fuse before lin: linearity)
                    pgf = ps_sm2.tile([K, 512], F32, tag="sm")
                    for g in range(3):
                        nc.tensor.matmul(pgf[:],
                                         fuses[:114, K * g:K * (g + 1)],
                                         gstack[g][:],
                                         start=(g == 0), stop=(g == 2))
                    nc.scalar.copy(gf_sb[:], pgf[:])

                    # spread stationary transposes: one [128,76] PE
                    # transpose per (bim, cc) covers all 4 bins (cols
                    # 19j+k of m=4bim+j land at g_ct col 19m)
                    for bim in range(4):
                        for cc in range(4):
                            pt = ps_t2.tile([128, 76], BF16, tag="t2")
                            nc.tensor.transpose(
                                pt[:],
                                lg_bf[:, 512 * bim + 128 * cc:
                                      512 * bim + 128 * (cc + 1)],
                                si76[:])
                            nc.scalar.copy(
                                g_ct[:, 304 * cc + 76 * bim:
                                     304 * cc + 76 * (bim + 1)], pt[:])
                    for cc in range(4):
                        pt = ps_t2.tile([128, K], BF16, tag="t2")
                        nc.tensor.transpose(
                            pt[:], gf_sb[:, 128 * cc:128 * (cc + 1)],
                            i128[:K, :K])
                        nc.scalar.copy(gf_ct[:, K * cc:K * (cc + 1)], pt[:])

                    # local_g = g @ lin_w^T : [128,(dchunk,m,k)]
                    for ddc in range(4):
                        plg = ps_mm2.tile([128, 304], F32, tag="mm2")
                        for cc in range(4):
                            nc.tensor.matmul(
                                plg[:],
                                linw[:, 512 * cc + 128 * ddc:
                                     512 * cc + 128 * ddc + 128],
                                g_ct[:, 304 * cc:304 * (cc + 1)],
                                start=(cc == 0), stop=(cc == 3))
                        nc.scalar.copy(localg_ct[:, 304 * ddc:304 * (ddc + 1)],
                                       plg[:])

                    # kk = local_g @ k_w^T + k_b -> bf16 [128,(di2, m, k)]
                    for di in range(2):
                        pkk = ps_mm2.tile([128, 304], F32, tag="mm2")
                        for cc in range(4):
                            nc.tensor.matmul(
                                pkk[:],
                                kw[:, 256 * cc + 128 * di:
                                   256 * cc + 128 * di + 128],
                                localg_ct[:, 304 * cc:304 * (cc + 1)],
                                start=(cc == 0), stop=(cc == 3))
                        nc.scalar.activation(
                            kk_sb[:, 304 * di:304 * (di + 1)], pkk[:],
                            AF.Identity, bias=kb[:, di:di + 1], scale=1.0)

                    # glob = prelu(gf @ lin_w^T + fuse_b) -> [128,(cchunk4,k)]
                    for ddc in range(4):
                        pgl = ps_sm2.tile([128, K], F32, tag="smg")
                        for cc in range(4):
                            nc.tensor.matmul(
                                pgl[:],
                                linw[:, 512 * cc + 128 * ddc:
                                     512 * cc + 128 * ddc + 128],
                                gf_ct[:, K * cc:K * (cc + 1)],
                                start=(cc == 0), stop=(cc == 3))
                        gz = glob_ct[:, K * ddc:K * (ddc + 1)]
                        nc.scalar.activation(gz, pgl[:], AF.Identity,
                                             bias=fb[:], scale=1.0)
                        gtmp = gpool.tile([128, K], BF16, tag="gtmp",
                                          name=f"gtmp{ddc}")
                        nc.vector.tensor_scalar(
                            gtmp[:], gz, 0.0, fam1[:],
                            op0=ALU.min, op1=ALU.mult)
                        nc.vector.tensor_add(gz, gz, gtmp[:])

                    # v = glob @ v_w^T + v_b : [19, 256] bf16
                    pv = ps_sm2.tile([K, 512], F32, tag="sm")
                    for cc in range(4):
                        nc.tensor.matmul(
                            pv[:, :256], glob_ct[:, K * cc:K * (cc + 1)],
                            vw[:, 256 * cc:256 * (cc + 1)],
                            start=(cc == 0), stop=(cc == 3))
                    nc.vector.tensor_add(v_sb[:], pv[:, :256], vb[:])

        # =================== PASS 2 ===================
        tc.strict_bb_all_engine_barrier()
        q_v = q_sb[:].rearrange("p (d n w) -> p d n w", d=2, n=16)
        with tc.tile_pool(name="osb", bufs=2) as o_pool, \
             tc.tile_pool(name="eaff", bufs=2) as ea_pool, \
             tc.tile_pool(name="ssb", bufs=2) as s_pool, \
             tc.tile_pool(name="sinvb", bufs=2) as si_pool, \
             tc.tile_pool(name="xr", bufs=3) as xr_pool, \
             tc.tile_pool(name="wsb", bufs=3) as w_pool, \
             tc.tile_pool(name="ps_aff", bufs=2, space="PSUM") as ps_aff, \
             tc.tile_pool(name="ps_sp", bufs=1, space="PSUM") as ps_sp, \
             tc.tile_pool(name="ps_sb", bufs=1, space="PSUM") as ps_sb, \
             tc.tile_pool(name="ps_o", bufs=2, space="PSUM") as ps_o, \
             tc.tile_pool(name="ps_y", bufs=2, space="PSUM") as ps_y:
            for bi in range(4):
                # x rows for the residual: quarters still resident from
                # pass 1 are reused in place; older ones are prefetched
                # (bf16, cast in flight, one batched DMA per quarter)
                xr_tiles = {}
                for qq in range(4):
                    if 4 * bi + qq >= 16 - XQ_BUFS:
                        xr_tiles[qq] = xq_keep[(bi, qq)]
                        continue
                    r0 = 32 * bi + 8 * qq
                    xr = xr_pool.tile([128, 4096], BF16, tag="xr",
                                      name="xr")
                    nc.gpsimd.dma_start(
                        out=xr[:].rearrange("p (c a b) -> p c a b",
                                            c=4, a=8),
                        in_=x_d[:].rearrange("(c p) h w -> p c h w",
                                             c=4)[:, :, r0:r0 + 8, :])
                    xr_tiles[qq] = xr
                # --- 2A: attention per bin ---
                # o stored fp8 (x16 scale folded into v_w/v_b on the
                # host), contraction-pair interleaved for the DoubleRow
                # out-conv: element (j, px, di) at col 2048j + 2px + di
                o_sb = o_pool.tile([128, 2 * 4 * PBIN], FP8, tag="osb")
                for j in range(4):
                    n = 4 * bi + j
                    eaff = ea_pool.tile([K, PBIN], BF16, tag="eaff")
                    s_sb = s_pool.tile([1, PBIN], BF16, tag="ssb")
                    for hh in range(2):
                        sinv = si_pool.tile([128, 512], F32, tag="sinvb")
                        pa = ps_aff.tile([K, 512], F32, tag="aff")
                        for di in range(2):
                            nc.tensor.matmul(
                                pa[:],
                                kk_sb[:, 304 * di + K * n:
                                      304 * di + K * (n + 1)],
                                q_v[:, di, n, 512 * hh:512 * (hh + 1)],
                                start=(di == 0), stop=(di == 1))
                        nc.scalar.activation(
                            eaff[:, 512 * hh:512 * (hh + 1)], pa[:],
                            AF.Exp, bias=0.0, scale=1.0)
                        psx = ps_sp.tile([1, 512], F32, tag="sp")
                        nc.tensor.matmul(psx[:], ones19[:],
                                         eaff[:, 512 * hh:512 * (hh + 1)],
                                         start=True, stop=True)
                        nc.scalar.copy(s_sb[:, 512 * hh:512 * (hh + 1)],
                                       psx[:])
                        pb = ps_sb.tile([128, 512], F32, tag="sb")
                        nc.tensor.matmul(pb[:], ones1[:],
                                         s_sb[:, 512 * hh:512 * (hh + 1)],
                                         start=True, stop=True)
                        nc.vector.reciprocal(sinv[:], pb[:])
                        for di in range(2):
                            po = ps_o.tile([128, 512], F32, tag="o")
                            nc.tensor.matmul(
                                po[:], v_sb[:, 128 * di:128 * (di + 1)],
                                eaff[:, 512 * hh:512 * (hh + 1)],
                                start=True, stop=True)
                            nc.vector.tensor_mul(
                                o_sb[:, 2048 * j + 1024 * hh + di:
                                     2048 * j + 1024 * hh + di + 1023:2],
                                po[:], sinv[:])
                # --- 2B: out conv + BN + prelu + residual per quarter-row --
                # bn scale is folded into out_wT on the host; here:
                # w = prelu(conv + bn_b, a) on ACT (in-place on PSUM),
                # then one DVE add for the residual
                for qq in range(4):
                    r0 = 32 * bi + 8 * qq
                    xr_t = xr_tiles[qq]
                    for cc in range(4):
                        xrv = xr_t[:, 1024 * cc:1024 * (cc + 1)].rearrange(
                            "p (a b) -> p a b", a=8)
                        for jp in range(2):      # bin pair (j0, j0+1)
                            j0 = 2 * jp
                            py = ps_y.tile([128, 512], F32, tag="y")
                            for dj in range(2):
                                j = j0 + dj
                                nc.tensor.matmul(
                                    py[:, 256 * dj:256 * (dj + 1)],
                                    outw8[:].rearrange(
                                        "p (i c) -> p i c", i=2)[
                                        :, :, 128 * cc:128 * (cc + 1)],
                                    o_sb[:, 2048 * j + 512 * qq:
                                         2048 * j + 512 * qq +
                                         512].rearrange(
                                        "p (w i) -> p i w", i=2),
                                    perf_mode=mybir.MatmulPerfMode.DoubleRow,
                                    start=True, stop=True)
                            w_sb = w_pool.tile([128, 512], BF16, tag="w",
                                               name="w_sb")
                            # deq = 1/(sw*so) dequantizes the fp8 conv
                            nc.scalar.activation(
                                w_sb[:], py[:], AF.Prelu,
                                bias=bnb[:, cc:cc + 1], scale=deq[:],
                                alpha=out_pa[:, cc:cc + 1])
                            # w free layout (j2, r8, w32) -> xr (r8, j2*w32)
                            wv = w_sb[:].rearrange(
                                "p (j r w) -> p r j w", j=2, r=8)
                            xrj = xrv[:, :, 32 * j0:32 * j0 + 64].rearrange(
                                "p r (j w) -> p r j w", j=2)
                            nc.vector.tensor_add(xrj, wv, xrj)
                    # one batched y write per quarter (casts bf16 -> f32)
                    nc.gpsimd.dma_start(
                        out=y_d[:].rearrange("(c p) h w -> p c h w",
                                             c=4)[:, :, r0:r0 + 8, :],
                        in_=xr_t[:].rearrange("p (c a b) -> p c a b",
                                              c=4, a=8))
    return nc


def split_excess_waits(nc, max_waits=1):
    """Walrus rejects instructions with more than `max_waits` sync-wait
    commands. Move excess waits onto preceding same-engine NoOps (engine
    queues are in-order, so this is semantics-preserving)."""
    n_split = 0
    for f in nc.m.functions:
        for blk in f.blocks:
            new = []
            for inst in blk.instructions:
                si = inst.sync_info
                if si is not None and si.on_wait and len(si.on_wait) > max_waits:
                    waits = list(si.on_wait)
                    k = 0
                    while len(waits) > max_waits:
                        chunk, waits = waits[:max_waits], waits[max_waits:]
                        nop = mybir.InstNoOp(
                            name=f"{inst.name}-ws{k}",
                            engine=inst.engine,
                            sync_info=mybir.SyncInfo(on_wait=chunk,
                                                     on_update=[]),
                            bass_nofuse=True,
                        )
                        new.append(nop)
                        k += 1
                        n_split += 1
                    inst.sync_info = mybir.SyncInfo(
                        on_wait=waits, on_update=list(si.on_update))
                new.append(inst)
            blk.instructions[:] = new
    return n_split


_NC_CACHE = {}


def get_nc():
    if "nc" not in _NC_CACHE:
        nc = build_nc()
        split_excess_waits(nc)
        _NC_CACHE["nc"] = nc
    return _NC_CACHE["nc"]


def prep_inputs(inputs):
    """Host-side re-layout of the module parameters (per-core, shared)."""
    f = lambda a: np.asarray(a, dtype=np.float32)
    bf = ml_dtypes.bfloat16
    conv_cam_w = f(inputs["conv_cam_w"])
    q_w, k_w, v_w = f(inputs["q_w"]), f(inputs["k_w"]), f(inputs["v_w"])
    lin_w = f(inputs["gcn_lin_w"])
    out_w = f(inputs["out_conv_w"])
    w1 = f(inputs["gcn_conv1_w"])
    fuse_w = f(inputs["fuse_w"])

    def chunkT(w, nchunk):  # [D, C] -> [128, (cchunk, D)]
        D = w.shape[0]
        return np.ascontiguousarray(
            w.T.reshape(nchunk, 128, D).transpose(1, 0, 2).reshape(
                128, nchunk * D))

    # w1s[19nn+i, 512g + 32jm + k] = W1[4bim+jm, 6g+nn] * (i==k), per bim
    w1s = np.zeros((128, 3, 4, 128), np.float32)
    fuse_s = np.zeros((128, 3 * K), np.float32)
    eye19 = np.eye(K, dtype=np.float32)
    for n in range(NBINS):
        g, nn = n // 6, n % 6
        for m in range(NBINS):
            bim, jm = m // 4, m % 4
            w1s[19 * nn:19 * nn + 19, g, bim,
                32 * jm:32 * jm + 19] = eye19 * w1[m, n]
        fuse_s[19 * nn:19 * nn + 19, K * g:K * (g + 1)] = eye19 * fuse_w[n]
    w1s = w1s.reshape(128, 3 * 512)

    # si19[32j + i, k] = (i == k) stacked identity
    si19 = np.zeros((128, K), np.float32)
    for j in range(4):
        si19[32 * j:32 * j + 19, :] = eye19
    # si76[32j + i, 19j + k] = (i == k): spread stacked identity
    si76 = np.zeros((128, 76), np.float32)
    for j in range(4):
        si76[32 * j:32 * j + 19, 19 * j:19 * j + 19] = eye19

    # gcn prelu alphas in stacked layout: row 32j+k, col bim -> a[4bim+j]-1
    gcn_am1 = np.zeros((128, 4), np.float32)
    ga = f(inputs["gcn_prelu_a"]) - 1.0
    for bim in range(4):
        for jm in range(4):
            gcn_am1[32 * jm:32 * jm + 32, bim] = ga[4 * bim + jm]

    inv = 1.0 / np.sqrt(f(inputs["bn_var"]) + 1e-5)
    bn_a = f(inputs["bn_gamma"]) * inv
    bn_b = f(inputs["bn_beta"]) - f(inputs["bn_mean"]) * bn_a
    out_w_bn = bn_a[:, None] * out_w  # fold BN scale into the conv weights

    # fp8 out-conv: weights quantized with scale sw, o with so (folded
    # into v_w/v_b); the Prelu input scale dequantizes by 1/(sw*so)
    SO = 16.0
    sw = float(2.0 ** np.floor(np.log2(224.0 / max(np.abs(out_w_bn).max(),
                                                   1e-30))))
    out_w8 = np.zeros((128, 1024), np.float32)
    for i in range(2):
        # out_w8[p, 512i + cout] = out_w_bn[cout, p + 128i] * sw
        out_w8[:, 512 * i:512 * (i + 1)] = (out_w_bn[:, 128 * i:128 * (i + 1)]
                                            * sw).T

    return {
        "cam_wT": chunkT(conv_cam_w, 4).astype(bf),
        "cam_b": f(inputs["conv_cam_b"]).reshape(K, 1),
        "q_wT": chunkT(q_w, 4).astype(bf),
        "k_wT": chunkT(k_w, 4).astype(bf),
        "v_wT": chunkT(v_w * SO, 4).astype(bf),
        "lin_wT": chunkT(lin_w, 4).astype(bf),
        "out_w8": out_w8.astype(ml_dtypes.float8_e4m3),
        "out_deq": np.full((128, 1), 1.0 / (sw * SO), np.float32),
        "w1s": w1s.astype(bf),
        "fuse_s": fuse_s.astype(bf),
        "i128": np.eye(128, dtype=np.float32).astype(bf),
        "si19": si19.astype(bf),
        "si76": si76.astype(bf),
        "ones19": np.ones((K, 1), bf),
        "ones1": np.ones((1, 128), bf),
        "qb_t": np.ascontiguousarray(f(inputs["q_b"]).reshape(2, 128).T),
        "kb_t": np.ascontiguousarray(f(inputs["k_b"]).reshape(2, 128).T),
        "vb_bc": np.tile(f(inputs["v_b"])[None, :] * SO, (K, 1)),
        "fuse_b_bc": np.full((128, 1), f(inputs["fuse_b"])[0], np.float32),
        "fuse_a_bc": np.full(
            (128, 1), f(inputs["fuse_prelu_a"])[0] - 1.0, np.float32),
        "gcn_am1": gcn_am1,
        "bn_b": np.ascontiguousarray(bn_b.reshape(4, 128).T),
        "out_pam1": np.ascontiguousarray(
            (f(inputs["out_prelu_a"]) - 1.0).reshape(4, 128).T),
        "out_pa": np.ascontiguousarray(
            f(inputs["out_prelu_a"]).reshape(4, 128).T),
    }


def kernel(**inputs):
    from concourse.bass_utils import run_bass_kernel_spmd
    nc = get_nc()
    params = prep_inputs(inputs)
    x = np.asarray(inputs["x"], dtype=np.float32)
    in_maps = [dict(params, x=np.ascontiguousarray(x[b]))
               for b in range(NCORES)]
    res = run_bass_kernel_spmd(nc, in_maps, list(range(NCORES)))
    return np.stack([res.results[b]["y"] for b in range(NCORES)], axis=0)



# revision 91
# speedup vs baseline: 1.3798x; 1.2763x over previous
"""Trainium2 Bass kernel for the CAAM sparse-attention module.

Data-parallel over batch B=8 across 8 NeuronCores (one image per core).
All parameters replicated. Matmul fabric runs in bf16 (fp32 PSUM
accumulation); the out-conv runs in fp8e4m3 with the DoubleRow perf
mode (per-tensor power-of-2 scales folded into v_w/v_b host-side and
dequantized by the Prelu input scale); softmax normalizers and biases
stay fp32; the residual path is bf16 (x re-read cast in flight, y
written through a bf16->f32 cast DMA).

Pass 1 streams x once as [128c-chunk, (4cc, 8 rows, 128 cols)] bf16
quarter tiles (one batched gpsimd cast-DMA each). Per image row the
per-bin pixel contraction (local = pixconf @ x_p) does 4 tiny diagonal
transposes of exp(cam) into a persistent zero-padded block-diagonal
PSUM stationary, then ONE matmul contracting all 128 pixels, with the
whole bin-row (32 rows) accumulating in a single PSUM bank. Per-bin-row
softmax/sigmoid normalizers, local scaling and the stacked GCN input
layout are folded into the pass-1 tail. BN+PReLU+residual is one ACT
Prelu (per-partition alpha) plus one DVE add per tile. Pass 2 reuses
the last 8 resident x quarter tiles for the residual (bin-rows 2-3)
and prefetches the rest.
"""

import os

os.environ.setdefault("JAX_COMPILATION_CACHE_DIR", "/tmp/jax_comp_cache")
os.environ.setdefault("MYCRO_LOCAL_CACHE", "1")

import numpy as np
import ml_dtypes

import concourse.bass as bass
import concourse.mybir as mybir
import concourse.tile as tile
from contextlib import ExitStack

dt = mybir.dt
F32 = dt.float32
BF16 = dt.bfloat16
FP8 = dt.float8e4
AX = mybir.AxisListType
AF = mybir.ActivationFunctionType
ALU = mybir.AluOpType

C, H, W, K, CI = 512, 128, 128, 19, 256
NBINS = 16          # 4x4 bins
PBIN = 1024         # 32*32 pixels per bin
NCORES = 8


def build_nc():
    nc = bass.Bass("TRN2", target_bir_lowering=False, debug=False)

    x_d = nc.declare_dram_parameter("x", [C, H, W], F32, isOutput=False)
    camw_d = nc.declare_dram_parameter("cam_wT", [128, 4 * K], BF16, isOutput=False)
    camb_d = nc.declare_dram_parameter("cam_b", [K, 1], F32, isOutput=False)
    qw_d = nc.declare_dram_parameter("q_wT", [128, 1024], BF16, isOutput=False)
    kw_d = nc.declare_dram_parameter("k_wT", [128, 1024], BF16, isOutput=False)
    vw_d = nc.declare_dram_parameter("v_wT", [128, 1024], BF16, isOutput=False)
    linw_d = nc.declare_dram_parameter("lin_wT", [128, 2048], BF16, isOutput=False)
    outw_d = nc.declare_dram_parameter("out_w8", [128, 1024], FP8, isOutput=False)
    deq_d = nc.declare_dram_parameter("out_deq", [128, 1], F32, isOutput=False)
    w1s_d = nc.declare_dram_parameter("w1s", [128, 3 * 512], BF16, isOutput=False)
    fuses_d = nc.declare_dram_parameter("fuse_s", [128, 3 * K], BF16, isOutput=False)
    i128_d = nc.declare_dram_parameter("i128", [128, 128], BF16, isOutput=False)
    si19_d = nc.declare_dram_parameter("si19", [128, K], BF16, isOutput=False)
    si76_d = nc.declare_dram_parameter("si76", [128, 76], BF16, isOutput=False)
    outpa_raw_d = nc.declare_dram_parameter("out_pa", [128, 4], F32, isOutput=False)
    ones19_d = nc.declare_dram_parameter("ones19", [K, 1], BF16, isOutput=False)
    ones1_d = nc.declare_dram_parameter("ones1", [1, 128], BF16, isOutput=False)
    qb_d = nc.declare_dram_parameter("qb_t", [128, 2], F32, isOutput=False)
    kb_d = nc.declare_dram_parameter("kb_t", [128, 2], F32, isOutput=False)
    vb_d = nc.declare_dram_parameter("vb_bc", [K, 256], F32, isOutput=False)
    fb_d = nc.declare_dram_parameter("fuse_b_bc", [128, 1], F32, isOutput=False)
    fa_d = nc.declare_dram_parameter("fuse_a_bc", [128, 1], F32, isOutput=False)
    gcna_d = nc.declare_dram_parameter("gcn_am1", [128, 4], F32, isOutput=False)
    bnb_d = nc.declare_dram_parameter("bn_b", [128, 4], F32, isOutput=False)
    outpa_d = nc.declare_dram_parameter("out_pam1", [128, 4], F32, isOutput=False)
    y_d = nc.declare_dram_parameter("y", [C, H, W], F32, isOutput=True)

    with tile.TileContext(nc) as tc, ExitStack() as ctx:
        # ---------------- persistent SBUF ----------------
        cpool = ctx.enter_context(tc.tile_pool(name="consts", bufs=1))

        def load(dram, shape, dtype=F32, tag=None):
            t = cpool.tile(shape, dtype, tag=tag, name=tag)
            nc.sync.dma_start(out=t[:], in_=dram[:])
            return t

        camw = load(camw_d, [128, 4 * K], BF16, tag="camw")
        camb = load(camb_d, [K, 1], tag="camb")
        qw = load(qw_d, [128, 1024], BF16, tag="qw")
        kw = load(kw_d, [128, 1024], BF16, tag="kw")
        vw = load(vw_d, [128, 1024], BF16, tag="vw")
        linw = load(linw_d, [128, 2048], BF16, tag="linw")
        outw8 = load(outw_d, [128, 1024], FP8, tag="outw8")
        deq = load(deq_d, [128, 1], tag="deq")
        w1s = load(w1s_d, [128, 3 * 512], BF16, tag="w1s")
        fuses = load(fuses_d, [128, 3 * K], BF16, tag="fuses")
        i128 = load(i128_d, [128, 128], BF16, tag="i128")
        si19 = load(si19_d, [128, K], BF16, tag="si19")
        si76 = load(si76_d, [128, 76], BF16, tag="si76")
        ones19 = load(ones19_d, [K, 1], BF16, tag="ones19")
        ones1 = load(ones1_d, [1, 128], BF16, tag="ones1")
        qb = load(qb_d, [128, 2], tag="qb")
        kb = load(kb_d, [128, 2], tag="kb")
        vb = load(vb_d, [K, 256], tag="vb")
        fb = load(fb_d, [128, 1], tag="fb")
        fam1 = load(fa_d, [128, 1], tag="fam1")
        gcnam1 = load(gcna_d, [128, 4], tag="gcnam1")
        bnb = load(bnb_d, [128, 4], tag="bnb")
        pam1 = load(outpa_d, [128, 4], tag="pam1")
        out_pa = load(outpa_raw_d, [128, 4], tag="outpa")

        # x quarter tiles: pool spans both passes; the last XQ_BUFS
        # quarters from pass 1 stay resident and pass 2 (reverse order)
        # reuses them for the residual instead of re-reading x
        XQ_BUFS = 8
        xq_pool = ctx.enter_context(tc.tile_pool(name="xq", bufs=XQ_BUFS))

        ppool = ctx.enter_context(tc.tile_pool(name="persist", bufs=1))
        # q in bf16, bin-major: [128 dpart, (2 dchunk, 16 bin, 1024 px)]
        q_sb = ppool.tile([128, 2 * H * W], BF16, tag="q")
        kk_sb = ppool.tile([128, 2 * 304], BF16, tag="kk")
        v_sb = ppool.tile([K, 256], BF16, tag="vsb")
        scale_v2 = ppool.tile([128, 4], F32, tag="scalev2")
        locg = [ppool.tile([114, 512], BF16, tag=f"locg{g}",
                           name=f"locg{g}") for g in range(3)]
        gstack = [ppool.tile([114, 512], BF16, tag=f"gst{g}",
                             name=f"gst{g}") for g in range(3)]

        with tc.tile_pool(name="p1acc", bufs=1) as acc_pool:
            # stacked local sums: row 32j+k = bin(4bi+j) class k, col
            # (bi, c): [128, (4 binrow, 512 c)]
            local_all = acc_pool.tile([128, 4 * C], F32, tag="localall")
            lg_bf = acc_pool.tile([128, 4 * C], BF16, tag="lgbf")
            s_parts = acc_pool.tile([K, 128], F32, tag="sparts")
            cls_parts = acc_pool.tile([K, 128], F32, tag="clsparts")
            s_tot = acc_pool.tile([K, 16], F32, tag="stot")
            cls_sig = acc_pool.tile([K, 16], F32, tag="cls")
            scale_t = acc_pool.tile([K, 16], F32, tag="scalet")
            nc.vector.memset(scale_v2[:], 0.0)
            nc.vector.memset(locg[2][:], 0.0)
            nc.vector.memset(gstack[2][:], 0.0)

            # =================== PASS 1 ===================
            xq_keep = {}
            with tc.tile_pool(name="esb", bufs=3) as e_pool, \
                 tc.tile_pool(name="xtsb", bufs=10) as xt_pool, \
                 tc.tile_pool(name="ps_xt", bufs=1, space="PSUM") as ps_xt, \
                 tc.tile_pool(name="etsb", bufs=10) as et_pool, \
                 tc.tile_pool(name="ps_cam", bufs=2, space="PSUM") as ps_cam, \
                 tc.tile_pool(name="ps_q", bufs=2, space="PSUM") as ps_q, \
                 tc.tile_pool(name="ps_et", bufs=1, space="PSUM") as ps_et, \
                 tc.tile_pool(name="ps_loc", bufs=1, space="PSUM") as ps_loc:
                # persistent PSUM pair for the block-diagonal E_T; the
                # diagonal [32,19] blocks are rewritten by transposes,
                # the off-diagonal zeros from this one-time memset
                # persist for the whole pass (both buffers share one
                # PSUM bank: allocation is bank-granular)
                pet_bank = ps_et.tile([128, 1024], BF16, tag="pet",
                                      name="pet")
                pxt_bank = ps_xt.tile([128, 2048], BF16, tag="pxt",
                                      name="pxt")
                pxt2 = [pxt_bank[:, 512 * i:512 * (i + 1)]
                        for i in range(4)]
                pet2 = [pet_bank[:, 128 * i:128 * (i + 1)]
                        for i in range(4)]
                # PSUM can't be memset in bf16; zero it with a
                # transpose whose stationary operand is all zeros
                zcol = cpool.tile([1, 128], BF16, tag="zcol", name="zcol")
                zrow = cpool.tile([1, 256], BF16, tag="zrow", name="zrow")
                nc.vector.memset(zcol[:], 0.0)
                nc.vector.memset(zrow[:], 0.0)
                for i in range(4):
                    nc.tensor.transpose(
                        pet_bank[:, 256 * i:256 * (i + 1)], zcol[:],
                        zrow[:])
                for bi in range(4):          # bin-row
                    # stacked local accumulator for this bin-row: all 32
                    # image rows (4 quarters x 8) accumulate in PSUM
                    pl = ps_loc.tile([128, 512], F32, tag="loc")
                    for qq in range(4):      # quarter (8 image rows)
                        r0 = 32 * bi + 8 * qq
                        # one batched DMA for all 4 c-chunks of the
                        # quarter (gpsimd DMA casts f32 -> bf16 in
                        # flight; fewer descriptors = less Pool time)
                        xq_t = xq_pool.tile([128, 4096], BF16, tag="xq",
                                            name="xq")
                        nc.gpsimd.dma_start(
                            out=xq_t[:].rearrange("p (c a b) -> p c a b",
                                                  c=4, a=8),
                            in_=x_d[:].rearrange("(c p) h w -> p c h w",
                                                 c=4)[:, :, r0:r0 + 8, :])
                        xq_keep[(bi, qq)] = xq_t
                        xq = [xq_t[:, 1024 * cc:1024 * (cc + 1)]
                              for cc in range(4)]

                        e_sb = e_pool.tile([K, PBIN], BF16, tag="esb")
                        e_v = e_sb[:].rearrange("p (a b) -> p a b", a=8)
                        base = 32 * bi + 2 * qq
                        # cam + exp + per-bin-slot sums (one wide exp per
                        # half; slot sums via strided 4D reduces)
                        for hh in range(2):
                            pc = ps_cam.tile([K, 512], F32, tag="cam")
                            for cc in range(4):
                                nc.tensor.matmul(
                                    pc[:], camw[:, K * cc:K * (cc + 1)],
                                    xq[cc][:, 512 * hh:512 * (hh + 1)],
                                    start=(cc == 0), stop=(cc == 3))
                            pcv = pc[:].rearrange("p (a b) -> p a b", a=4)
                            nc.scalar.activation(
                                e_v[:, 4 * hh:4 * hh + 4, :], pcv,
                                AF.Exp, bias=camb[:], scale=1.0)
                            nc.vector.reduce_sum(
                                out=cls_parts[:, base + hh:base + hh + 25:8],
                                in_=pc[:].rearrange(
                                    "p (r j w) -> p j r w", r=4, j=4),
                                axis=AX.XY)
                            nc.vector.reduce_sum(
                                out=s_parts[:, base + hh:base + hh + 25:8],
                                in_=e_sb[:].rearrange(
                                    "p (r j w) -> p j r w", r=8, j=4)[
                                    :, :, 4 * hh:4 * hh + 4, :],
                                axis=AX.XY)

                        # q projection (written bin-major) -- emitted
                        # before the local matmuls so the PE has queued
                        # work while the xts DMA transposes land
                        for dd in range(2):
                            for hh in range(2):
                                pq = ps_q.tile([128, 512], F32, tag="q")
                                for cc in range(4):
                                    nc.tensor.matmul(
                                        pq[:],
                                        qw[:, 256 * cc + 128 * dd:
                                           256 * cc + 128 * dd + 128],
                                        xq[cc][:, 512 * hh:512 * (hh + 1)],
                                        start=(cc == 0), stop=(cc == 3))
                                pqv = pq[:].rearrange(
                                    "p (r j w) -> p j r w", r=4, j=4)
                                qdst = q_sb[:].rearrange(
                                    "p (d n w) -> p d n w", d=2, n=16)[
                                    :, dd, 4 * bi:4 * bi + 4,
                                    256 * qq + 128 * hh:
                                    256 * qq + 128 * hh + 128].rearrange(
                                    "p j (r w) -> p j r w", r=4)
                                nc.scalar.activation(
                                    qdst, pqv, AF.Identity,
                                    bias=qb[:, dd:dd + 1], scale=1.0)

                        # per image row: block-diag E_T (4 tiny diagonal
                        # transposes into the persistent zero-padded
                        # PSUM tile, copied out), then the 8 local
                        # matmuls contracting 128 pixels each
                        ets = []
                        xts = []
                        for rr in range(8):  # image row within quarter
                            pet = pet2[rr % 4]
                            pxt = pxt2[rr % 4]
                            for cc in range(4):
                                nc.tensor.transpose(
                                    pxt[:, 128 * cc:128 * (cc + 1)],
                                    xq[cc][:, 128 * rr:128 * (rr + 1)],
                                    i128[:])
                            xt_sb = xt_pool.tile([128, 512], BF16,
                                                 tag="xt", name="xt_sb")
                            if rr % 2 == 0:
                                nc.scalar.copy(xt_sb[:], pxt)
                            else:
                                nc.vector.tensor_copy(xt_sb[:], pxt)
                            xts.append(xt_sb)
                            for j in range(4):
                                nc.tensor.transpose(
                                    pet[32 * j:32 * j + 32,
                                        32 * j:32 * j + K],
                                    e_sb[:, 128 * rr + 32 * j:
                                         128 * rr + 32 * j + 32],
                                    i128[:K, :K],
                                    tile_position=(0, 32 * j))
                            et_sb = et_pool.tile([128, 128], BF16,
                                                 tag="et", name="et_sb")
                            nc.vector.tensor_copy(et_sb[:], pet)
                            ets.append(et_sb)
                        for rr in range(8):
                            nc.tensor.matmul(
                                pl[:], ets[rr][:], xts[rr][:],
                                start=(qq == 0 and rr == 0),
                                stop=(qq == 3 and rr == 7))
                    nc.vector.tensor_copy(
                        local_all[:, 512 * bi:512 * (bi + 1)], pl[:])

                    # per-bin-row normalizers + local scaling + stacked
                    # GCN input layout, folded into pass 1's tail so the
                    # GCN phase starts with its matmuls immediately
                    b4 = slice(4 * bi, 4 * bi + 4)
                    nc.vector.reduce_sum(
                        out=s_tot[:, b4],
                        in_=s_parts[:, 32 * bi:32 * bi + 32].rearrange(
                            "p (n q) -> p n q", n=4),
                        axis=AX.X)
                    nc.vector.reduce_sum(
                        out=cls_sig[:, b4],
                        in_=cls_parts[:, 32 * bi:32 * bi + 32].rearrange(
                            "p (n q) -> p n q", n=4),
                        axis=AX.X)
                    nc.scalar.activation(cls_sig[:, b4], cls_sig[:, b4],
                                         AF.Sigmoid, bias=camb[:],
                                         scale=1.0 / PBIN)
                    nc.vector.reciprocal(s_tot[:, b4], s_tot[:, b4])
                    nc.vector.tensor_mul(scale_t[:, b4], cls_sig[:, b4],
                                         s_tot[:, b4])
                    # scale_v2[32j+k, bi] = scale_t[k, 4bi+j]
                    for j in range(4):
                        nc.sync.dma_start(
                            out=scale_v2[32 * j:32 * j + K, bi:bi + 1],
                            in_=scale_t[:, 4 * bi + j:4 * bi + j + 1])
                    nc.vector.tensor_scalar_mul(
                        local_all[:, 512 * bi:512 * (bi + 1)],
                        local_all[:, 512 * bi:512 * (bi + 1)],
                        scale_v2[:, bi:bi + 1])
                    nc.vector.tensor_copy(
                        lg_bf[:, 512 * bi:512 * (bi + 1)],
                        local_all[:, 512 * bi:512 * (bi + 1)])
                    for j in range(4):
                        n = 4 * bi + j
                        g, mm = n // 6, n % 6
                        nc.sync.dma_start(
                            out=locg[g][19 * mm:19 * mm + 19, :],
                            in_=lg_bf[32 * j:32 * j + K,
                                      512 * bi:512 * (bi + 1)])

            # =================== GCN ===================
            with tc.tile_pool(name="gcn", bufs=1) as gpool:
                # GCN mix into the same stacked layout; overwrites
                # local_all in place. prelu(z,a) = z + (a-1)*min(z,0)
                with tc.tile_pool(name="ps_g", bufs=2, space="PSUM") as ps_g, \
                     tc.tile_pool(name="ptmp", bufs=2) as pt_pool:
                    for bim in range(4):
                        pg = ps_g.tile([128, 512], F32, tag="g")
                        for g in range(3):
                            nc.tensor.matmul(
                                pg[:],
                                w1s[:114, 512 * g + 128 * bim:
                                    512 * g + 128 * (bim + 1)],
                                locg[g][:], start=(g == 0), stop=(g == 2))
                        z = local_all[:, 512 * bim:512 * (bim + 1)]
                        nc.vector.tensor_add(z, pg[:], z)
                        ptmp = pt_pool.tile([128, 512], F32, tag="ptmp")
                        nc.vector.tensor_scalar(
                            ptmp[:], z, 0.0, gcnam1[:, bim:bim + 1],
                            op0=ALU.min, op1=ALU.mult)
                        nc.vector.tensor_add(z, z, ptmp[:])
                        nc.vector.tensor_copy(
                            lg_bf[:, 512 * bim:512 * (bim + 1)], z)
                        for jm in range(4):
                            m = 4 * bim + jm
                            g, mm = m // 6, m % 6
                            nc.sync.dma_start(
                                out=gstack[g][19 * mm:19 * mm + 19, :],
                                in_=lg_bf[32 * jm:32 * jm + K,
                                          512 * bim:512 * (bim + 1)])

                # transpose g -> c-partition layout [128,(cchunk4, m16, k19)]
                g_ct = gpool.tile([128, 4 * 304], BF16, tag="gct")
                gf_sb = gpool.tile([K, 512], BF16, tag="gfsb")
                gf_ct = gpool.tile([128, 4 * K], BF16, tag="gfct")
                localg_ct = gpool.tile([128, 4 * 304], BF16, tag="lgct")
                glob_ct = gpool.tile([128, 4 * K], BF16, tag="glob")

                with tc.tile_pool(name="ps_t2", bufs=2, space="PSUM") as ps_t2, \
                     tc.tile_pool(name="ps_mm2", bufs=2, space="PSUM") as ps_mm2, \
                     tc.tile_pool(name="ps_sm2", bufs=2, space="PSUM") as ps_sm2:
                    # gf = sum_n fuse_w[n] g[n]  (fuse before lin: linearity)
                    pgf = ps_sm2.tile([K, 512], F32, tag="sm")
                    for g in range(3):
                        nc.tensor.matmul(pgf[:],
                                         fuses[:114, K * g:K * (g + 1)],
                                         gstack[g][:],
                                         start=(g == 0), stop=(g == 2))
                    nc.scalar.copy(gf_sb[:], pgf[:])

                    # spread stationary transposes: one [128,76] PE
                    # transpose per (bim, cc) covers all 4 bins (cols
                    # 19j+k of m=4bim+j land at g_ct col 19m)
                    for bim in range(4):
                        for cc in range(4):
                            pt = ps_t2.tile([128, 76], BF16, tag="t2")
                            nc.tensor.transpose(
                                pt[:],
                                lg_bf[:, 512 * bim + 128 * cc:
                                      512 * bim + 128 * (cc + 1)],
                                si76[:])
                            nc.scalar.copy(
                                g_ct[:, 304 * cc + 76 * bim:
                                     304 * cc + 76 * (bim + 1)], pt[:])
                    for cc in range(4):
                        pt = ps_t2.tile([128, K], BF16, tag="t2")
                        nc.tensor.transpose(
                            pt[:], gf_sb[:, 128 * cc:128 * (cc + 1)],
                            i128[:K, :K])
                        nc.scalar.copy(gf_ct[:, K * cc:K * (cc + 1)], pt[:])

                    # local_g = g @ lin_w^T : [128,(dchunk,m,k)]
                    for ddc in range(4):
                        plg = ps_mm2.tile([128, 304], F32, tag="mm2")
                        for cc in range(4):
                            nc.tensor.matmul(
                                plg[:],
                                linw[:, 512 * cc + 128 * ddc:
                                     512 * cc + 128 * ddc + 128],
                                g_ct[:, 304 * cc:304 * (cc + 1)],
                                start=(cc == 0), stop=(cc == 3))
                        nc.scalar.copy(localg_ct[:, 304 * ddc:304 * (ddc + 1)],
                                       plg[:])

                    # kk = local_g @ k_w^T + k_b -> bf16 [128,(di2, m, k)]
                    for di in range(2):
                        pkk = ps_mm2.tile([128, 304], F32, tag="mm2")
                        for cc in range(4):
                            nc.tensor.matmul(
                                pkk[:],
                                kw[:, 256 * cc + 128 * di:
                                   256 * cc + 128 * di + 128],
                                localg_ct[:, 304 * cc:304 * (cc + 1)],
                                start=(cc == 0), stop=(cc == 3))
                        nc.scalar.activation(
                            kk_sb[:, 304 * di:304 * (di + 1)], pkk[:],
                            AF.Identity, bias=kb[:, di:di + 1], scale=1.0)

                    # glob = prelu(gf @ lin_w^T + fuse_b) -> [128,(cchunk4,k)]
                    for ddc in range(4):
                        pgl = ps_sm2.tile([128, K], F32, tag="smg")
                        for cc in range(4):
                            nc.tensor.matmul(
                                pgl[:],
                                linw[:, 512 * cc + 128 * ddc:
                                     512 * cc + 128 * ddc + 128],
                                gf_ct[:, K * cc:K * (cc + 1)],
                                start=(cc == 0), stop=(cc == 3))
                        gz = glob_ct[:, K * ddc:K * (ddc + 1)]
                        nc.scalar.activation(gz, pgl[:], AF.Identity,
                                             bias=fb[:], scale=1.0)
                        gtmp = gpool.tile([128, K], BF16, tag="gtmp",
                                          name=f"gtmp{ddc}")
                        nc.vector.tensor_scalar(
                            gtmp[:], gz, 0.0, fam1[:],
                            op0=ALU.min, op1=ALU.mult)
                        nc.vector.tensor_add(gz, gz, gtmp[:])

                    # v = glob @ v_w^T + v_b : [19, 256] bf16
                    pv = ps_sm2.tile([K, 512], F32, tag="sm")
                    for cc in range(4):
                        nc.tensor.matmul(
                            pv[:, :256], glob_ct[:, K * cc:K * (cc + 1)],
                            vw[:, 256 * cc:256 * (cc + 1)],
                            start=(cc == 0), stop=(cc == 3))
                    nc.vector.tensor_add(v_sb[:], pv[:, :256], vb[:])

        # =================== PASS 2 ===================
        tc.strict_bb_all_engine_barrier()
        q_v = q_sb[:].rearrange("p (d n w) -> p d n w", d=2, n=16)
        with tc.tile_pool(name="osb", bufs=2) as o_pool, \
             tc.tile_pool(name="eaff", bufs=2) as ea_pool, \
             tc.tile_pool(name="ssb", bufs=2) as s_pool, \
             tc.tile_pool(name="sinvb", bufs=2) as si_pool, \
             tc.tile_pool(name="xr", bufs=3) as xr_pool, \
             tc.tile_pool(name="wsb", bufs=3) as w_pool, \
             tc.tile_pool(name="ps_aff", bufs=2, space="PSUM") as ps_aff, \
             tc.tile_pool(name="ps_sp", bufs=1, space="PSUM") as ps_sp, \
             tc.tile_pool(name="ps_sb", bufs=1, space="PSUM") as ps_sb, \
             tc.tile_pool(name="ps_o", bufs=2, space="PSUM") as ps_o, \
             tc.tile_pool(name="ps_y", bufs=2, space="PSUM") as ps_y:
            for bi in range(4):
                # x rows for the residual: quarters still resident from
                # pass 1 are reused in place; older ones are prefetched
                # (bf16, cast in flight, one batched DMA per quarter)
                xr_tiles = {}
                for qq in range(4):
                    if 4 * bi + qq >= 16 - XQ_BUFS:
                        xr_tiles[qq] = xq_keep[(bi, qq)]
                        continue
                    r0 = 32 * bi + 8 * qq
                    xr = xr_pool.tile([128, 4096], BF16, tag="xr",
                                      name="xr")
                    nc.gpsimd.dma_start(
                        out=xr[:].rearrange("p (c a b) -> p c a b",
                                            c=4, a=8),
                        in_=x_d[:].rearrange("(c p) h w -> p c h w",
                                             c=4)[:, :, r0:r0 + 8, :])
                    xr_tiles[qq] = xr
                # --- 2A: attention per bin ---
                # o stored fp8 (x16 scale folded into v_w/v_b on the
                # host), contraction-pair interleaved for the DoubleRow
                # out-conv: element (j, px, di) at col 2048j + 2px + di
                o_sb = o_pool.tile([128, 2 * 4 * PBIN], FP8, tag="osb")
                for j in range(4):
                    n = 4 * bi + j
                    eaff = ea_pool.tile([K, PBIN], BF16, tag="eaff")
                    s_sb = s_pool.tile([1, PBIN], BF16, tag="ssb")
                    for hh in range(2):
                        sinv = si_pool.tile([128, 512], F32, tag="sinvb")
                        pa = ps_aff.tile([K, 512], F32, tag="aff")
                        for di in range(2):
                            nc.tensor.matmul(
                                pa[:],
                                kk_sb[:, 304 * di + K * n:
                                      304 * di + K * (n + 1)],
                                q_v[:, di, n, 512 * hh:512 * (hh + 1)],
                                start=(di == 0), stop=(di == 1))
                        nc.scalar.activation(
                            eaff[:, 512 * hh:512 * (hh + 1)], pa[:],
                            AF.Exp, bias=0.0, scale=1.0)
                        psx = ps_sp.tile([1, 512], F32, tag="sp")
                        nc.tensor.matmul(psx[:], ones19[:],
                                         eaff[:, 512 * hh:512 * (hh + 1)],
                                         start=True, stop=True)
                        nc.scalar.copy(s_sb[:, 512 * hh:512 * (hh + 1)],
                                       psx[:])
                        pb = ps_sb.tile([128, 512], F32, tag="sb")
                        nc.tensor.matmul(pb[:], ones1[:],
                                         s_sb[:, 512 * hh:512 * (hh + 1)],
                                         start=True, stop=True)
                        nc.vector.reciprocal(sinv[:], pb[:])
                        for di in range(2):
                            po = ps_o.tile([128, 512], F32, tag="o")
                            nc.tensor.matmul(
                                po[:], v_sb[:, 128 * di:128 * (di + 1)],
                                eaff[:, 512 * hh:512 * (hh + 1)],
                                start=True, stop=True)
                            nc.vector.tensor_mul(
                                o_sb[:, 2048 * j + 1024 * hh + di:
                                     2048 * j + 1024 * hh + di + 1023:2],
                                po[:], sinv[:])
                # --- 2B: out conv + BN + prelu + residual per quarter-row --
                # bn scale is folded into out_wT on the host; here:
                # w = prelu(conv + bn_b, a) on ACT (in-place on PSUM),
                # then one DVE add for the residual
                for qq in range(4):
                    r0 = 32 * bi + 8 * qq
                    xr_t = xr_tiles[qq]
                    for cc in range(4):
                        xrv = xr_t[:, 1024 * cc:1024 * (cc + 1)].rearrange(
                            "p (a b) -> p a b", a=8)
                        for jp in range(2):      # bin pair (j0, j0+1)
                            j0 = 2 * jp
                            py = ps_y.tile([128, 512], F32, tag="y")
                            for dj in range(2):
                                j = j0 + dj
                                nc.tensor.matmul(
                                    py[:, 256 * dj:256 * (dj + 1)],
                                    outw8[:].rearrange(
                                        "p (i c) -> p i c", i=2)[
                                        :, :, 128 * cc:128 * (cc + 1)],
                                    o_sb[:, 2048 * j + 512 * qq:
                                         2048 * j + 512 * qq +
                                         512].rearrange(
                                        "p (w i) -> p i w", i=2),
                                    perf_mode=mybir.MatmulPerfMode.DoubleRow,
                                    start=True, stop=True)
                            w_sb = w_pool.tile([128, 512], BF16, tag="w",
                                               name="w_sb")
                            # deq = 1/(sw*so) dequantizes the fp8 conv
                            nc.scalar.activation(
                                w_sb[:], py[:], AF.Prelu,
                                bias=bnb[:, cc:cc + 1], scale=deq[:],
                                alpha=out_pa[:, cc:cc + 1])
                            # w free layout (j2, r8, w32) -> xr (r8, j2*w32)
                            wv = w_sb[:].rearrange(
                                "p (j r w) -> p r j w", j=2, r=8)
                            xrj = xrv[:, :, 32 * j0:32 * j0 + 64].rearrange(
                                "p r (j w) -> p r j w", j=2)
                            nc.vector.tensor_add(xrj, wv, xrj)
                    # one batched y write per quarter (casts bf16 -> f32)
                    nc.gpsimd.dma_start(
                        out=y_d[:].rearrange("(c p) h w -> p c h w",
                                             c=4)[:, :, r0:r0 + 8, :],
                        in_=xr_t[:].rearrange("p (c a b) -> p c a b",
                                              c=4, a=8))
    return nc


def split_excess_waits(nc, max_waits=1):
    """Walrus rejects instructions with more than `max_waits` sync-wait
    commands. Move excess waits onto preceding same-engine NoOps (engine
    queues are in-order, so this is semantics-preserving)."""
    n_split = 0
    for f in nc.m.functions:
        for blk in f.blocks:
            new = []
            for inst in blk.instructions:
                si = inst.sync_info
                if si is not None and si.on_wait and len(si.on_wait) > max_waits:
                    waits = list(si.on_wait)
                    k = 0
                    while len(waits) > max_waits:
                        chunk, waits = waits[:max_waits], waits[max_waits:]
                        nop = mybir.InstNoOp(
                            name=f"{inst.name}-ws{k}",
                            engine=inst.engine,
                            sync_info=mybir.SyncInfo(on_wait=chunk,
                                                     on_update=[]),
                            bass_nofuse=True,
                        )
                        new.append(nop)
                        k += 1
                        n_split += 1
                    inst.sync_info = mybir.SyncInfo(
                        on_wait=waits, on_update=list(si.on_update))
                new.append(inst)
            blk.instructions[:] = new
    return n_split


_NC_CACHE = {}


def get_nc():
    if "nc" not in _NC_CACHE:
        nc = build_nc()
        split_excess_waits(nc)
        _NC_CACHE["nc"] = nc
    return _NC_CACHE["nc"]


def prep_inputs(inputs):
    """Host-side re-layout of the module parameters (per-core, shared)."""
    f = lambda a: np.asarray(a, dtype=np.float32)
    bf = ml_dtypes.bfloat16
    conv_cam_w = f(inputs["conv_cam_w"])
    q_w, k_w, v_w = f(inputs["q_w"]), f(inputs["k_w"]), f(inputs["v_w"])
    lin_w = f(inputs["gcn_lin_w"])
    out_w = f(inputs["out_conv_w"])
    w1 = f(inputs["gcn_conv1_w"])
    fuse_w = f(inputs["fuse_w"])

    def chunkT(w, nchunk):  # [D, C] -> [128, (cchunk, D)]
        D = w.shape[0]
        return np.ascontiguousarray(
            w.T.reshape(nchunk, 128, D).transpose(1, 0, 2).reshape(
                128, nchunk * D))

    # w1s[19nn+i, 512g + 32jm + k] = W1[4bim+jm, 6g+nn] * (i==k), per bim
    w1s = np.zeros((128, 3, 4, 128), np.float32)
    fuse_s = np.zeros((128, 3 * K), np.float32)
    eye19 = np.eye(K, dtype=np.float32)
    for n in range(NBINS):
        g, nn = n // 6, n % 6
        for m in range(NBINS):
            bim, jm = m // 4, m % 4
            w1s[19 * nn:19 * nn + 19, g, bim,
                32 * jm:32 * jm + 19] = eye19 * w1[m, n]
        fuse_s[19 * nn:19 * nn + 19, K * g:K * (g + 1)] = eye19 * fuse_w[n]
    w1s = w1s.reshape(128, 3 * 512)

    # si19[32j + i, k] = (i == k) stacked identity
    si19 = np.zeros((128, K), np.float32)
    for j in range(4):
        si19[32 * j:32 * j + 19, :] = eye19
    # si76[32j + i, 19j + k] = (i == k): spread stacked identity
    si76 = np.zeros((128, 76), np.float32)
    for j in range(4):
        si76[32 * j:32 * j + 19, 19 * j:19 * j + 19] = eye19

    # gcn prelu alphas in stacked layout: row 32j+k, col bim -> a[4bim+j]-1
    gcn_am1 = np.zeros((128, 4), np.float32)
    ga = f(inputs["gcn_prelu_a"]) - 1.0
    for bim in range(4):
        for jm in range(4):
            gcn_am1[32 * jm:32 * jm + 32, bim] = ga[4 * bim + jm]

    inv = 1.0 / np.sqrt(f(inputs["bn_var"]) + 1e-5)
    bn_a = f(inputs["bn_gamma"]) * inv
    bn_b = f(inputs["bn_beta"]) - f(inputs["bn_mean"]) * bn_a
    out_w_bn = bn_a[:, None] * out_w  # fold BN scale into the conv weights

    # fp8 out-conv: weights quantized with scale sw, o with so (folded
    # into v_w/v_b); the Prelu input scale dequantizes by 1/(sw*so)
    SO = 16.0
    sw = float(2.0 ** np.floor(np.log2(224.0 / max(np.abs(out_w_bn).max(),
                                                   1e-30))))
    out_w8 = np.zeros((128, 1024), np.float32)
    for i in range(2):
        # out_w8[p, 512i + cout] = out_w_bn[cout, p + 128i] * sw
        out_w8[:, 512 * i:512 * (i + 1)] = (out_w_bn[:, 128 * i:128 * (i + 1)]
                                            * sw).T

    return {
        "cam_wT": chunkT(conv_cam_w, 4).astype(bf),
        "cam_b": f(inputs["conv_cam_b"]).reshape(K, 1),
        "q_wT": chunkT(q_w, 4).astype(bf),
        "k_wT": chunkT(k_w, 4).astype(bf),
        "v_wT": chunkT(v_w * SO, 4).astype(bf),
        "lin_wT": chunkT(lin_w, 4).astype(bf),
        "out_w8": out_w8.astype(ml_dtypes.float8_e4m3),
        "out_deq": np.full((128, 1), 1.0 / (sw * SO), np.float32),
        "w1s": w1s.astype(bf),
        "fuse_s": fuse_s.astype(bf),
        "i128": np.eye(128, dtype=np.float32).astype(bf),
        "si19": si19.astype(bf),
        "si76": si76.astype(bf),
        "ones19": np.ones((K, 1), bf),
        "ones1": np.ones((1, 128), bf),
        "qb_t": np.ascontiguousarray(f(inputs["q_b"]).reshape(2, 128).T),
        "kb_t": np.ascontiguousarray(f(inputs["k_b"]).reshape(2, 128).T),
        "vb_bc": np.tile(f(inputs["v_b"])[None, :] * SO, (K, 1)),
        "fuse_b_bc": np.full((128, 1), f(inputs["fuse_b"])[0], np.float32),
        "fuse_a_bc": np.full(
            (128, 1), f(inputs["fuse_prelu_a"])[0] - 1.0, np.float32),
        "gcn_am1": gcn_am1,
        "bn_b": np.ascontiguousarray(bn_b.reshape(4, 128).T),
        "out_pam1": np.ascontiguousarray(
            (f(inputs["out_prelu_a"]) - 1.0).reshape(4, 128).T),
        "out_pa": np.ascontiguousarray(
            f(inputs["out_prelu_a"]).reshape(4, 128).T),
    }


def kernel(**inputs):
    from concourse.bass_utils import run_bass_kernel_spmd
    nc = get_nc()
    params = prep_inputs(inputs)
    x = np.asarray(inputs["x"], dtype=np.float32)
    in_maps = [dict(params, x=np.ascontiguousarray(x[b]))
               for b in range(NCORES)]
    res = run_bass_kernel_spmd(nc, in_maps, list(range(NCORES)))
    return np.stack([res.results[b]["y"] for b in range(NCORES)], axis=0)

